# revision 14
# baseline (speedup 1.0000x reference)
"""Trainium2 Bass kernel for nn_DenseReluGMMConvNetwork (2-layer GMMConv GNN).

Self-contained: takes FULL inputs, shards nodes across 8 NeuronCores,
runs one SPMD Bass program (gather / GMM weights / scatter-matmul /
BN via AllReduce / inter-layer AllGather), returns FULL [50000, 64] output.
"""

import os
import sys

sys.path.insert(0, "/opt/trn_rl_repo")

import numpy as np

# ---- problem constants (overridable for small-scale sim tests) ----
N = 50000
E = 800000
D = 3
K = 4
C = 64
NCORES = 8
EPS = 1e-15
BN_EPS = 1e-5
SG_WINDOWS = 4          # windows per gather super-group
HALF_SPLIT = 32768      # int16 index range split
DEBUG_TAPS = False

LAST_RESULT = None


def _derived():
    npc = N // NCORES
    win = (npc + 127) // 128          # windows per core
    node_pad = win * 128              # padded rows per core
    trows = NCORES * node_pad         # padded gather-table rows
    return npc, win, node_pad, trows


def host_prep(edges):
    """Route + sort edges, build the uniform chunk grid and slot arrays.

    Returns dict with per-core routed arrays and the static chunk grid.
    """
    npc, win, node_pad, trows = _derived()
    src = np.asarray(edges[0], np.int64)
    dst = np.asarray(edges[1], np.int64)
    e = src.shape[0]

    core = dst // npc
    dl = dst - core * npc
    wi = dl >> 7
    dj = dl & 127
    prow = (src // npc) * node_pad + (src % npc)
    half = (prow >= HALF_SPLIT).astype(np.int64)

    # group key: (core, half, wi); lo region first per core
    gkey = (core * 2 + half) * win + wi
    order = np.argsort(gkey, kind="stable")
    cnt = np.bincount(gkey, minlength=NCORES * 2 * win)
    goff = np.zeros_like(cnt)
    goff[1:] = np.cumsum(cnt)[:-1]
    rank = np.arange(e) - goff[gkey[order]]         # rank within group (sorted order)

    cnt3 = cnt.reshape(NCORES, 2, win)
    ncl = (cnt3[:, 0, :].max(axis=0) + 127) // 128  # lo chunks per window (shared)
    nch = (cnt3[:, 1, :].max(axis=0) + 127) // 128  # hi chunks per window
    ncl = np.maximum(ncl, 1)
    nch = np.maximum(nch, 1)
    nchl_tot = int(ncl.sum())
    nchh_tot = int(nch.sum())
    ncht = nchl_tot + nchh_tot
    lo_off = np.zeros(win, np.int64)
    lo_off[1:] = np.cumsum(ncl)[:-1]
    hi_off = np.zeros(win, np.int64)
    hi_off[1:] = np.cumsum(nch)[:-1]
    hi_off += nchl_tot

    # slot index for each (sorted) edge
    so = order
    chunk_base = np.where(half[so] == 0, lo_off[wi[so]], hi_off[wi[so]])
    slot = chunk_base * 128 + rank

    nslots = ncht * 128
    idx16 = np.zeros((NCORES, nslots), np.int16)
    pseudo_slot = np.zeros((NCORES, nslots, D), np.float32)
    dstw = np.full((NCORES, nslots), -1.0, np.float32)

    cs = core[so]
    iv = prow[so] - half[so] * HALF_SPLIT
    idx16[cs, slot] = iv.astype(np.int16)
    dstw[cs, slot] = dj[so].astype(np.float32)

    deg = np.bincount(dst, minlength=N).astype(np.float32)
    invdeg_flat = 1.0 / np.maximum(deg, 1.0)
    invdeg = np.zeros((NCORES, 128, win), np.float32)
    for m in range(NCORES):
        v = np.zeros(node_pad, np.float32)
        v[:npc] = invdeg_flat[m * npc:(m + 1) * npc]
        invdeg[m] = v.reshape(win, 128).T

    return dict(
        order=order, slot=slot, core_sorted=cs,
        ncl=ncl, nch=nch, ncht=ncht, nchl_tot=nchl_tot,
        lo_off=lo_off, hi_off=hi_off,
        idx16=idx16, pseudo_slot=pseudo_slot, dstw=dstw, invdeg=invdeg,
        npc=npc, win=win, node_pad=node_pad, trows=trows,
    )


def fill_pseudo(prep, pseudo):
    ps = np.asarray(pseudo, np.float32)
    prep["pseudo_slot"][:] = 0.0
    prep["pseudo_slot"][prep["core_sorted"], prep["slot"]] = ps[prep["order"]]


def build_program(prep):
    import concourse.bacc as bacc
    import concourse.mybir as mybir
    import concourse.tile as tile
    from concourse.library_config import mlp

    f32 = mybir.dt.float32
    i16 = mybir.dt.int16
    AF = mybir.ActivationFunctionType
    OP = mybir.AluOpType

    win = prep["win"]
    node_pad = prep["node_pad"]
    trows = prep["trows"]
    ncht = prep["ncht"]
    ncl = prep["ncl"]
    nch = prep["nch"]
    lo_off = prep["lo_off"]
    hi_off = prep["hi_off"]
    nfull = float(N)

    # super-groups of windows
    sgs = [list(range(s, min(s + SG_WINDOWS, win))) for s in range(0, win, SG_WINDOWS)]

    nc = bacc.Bacc("TRN2", target_bir_lowering=False, num_devices=NCORES)

    def inp(name, shape, dt=f32):
        return nc.dram_tensor(name, shape, dt, kind="ExternalInput").ap()

    in_tab = inp("tab", [trows, C])
    in_idx = inp("idx", [128, ncht * 8], i16)
    in_ps = inp("pseudo", [128, ncht, D])
    in_dw = inp("dstw", [128, ncht])
    in_xT = inp("xT0", [C, node_pad])
    in_ivd = inp("invdeg", [128, win])
    in_iota = inp("iota", [128, 128])
    in_ident = inp("ident", [128, 128])
    in_ones = inp("onesv", [128, 2])          # col0: ones, col1: valid mask last window
    in_G = [inp(f"g{l}c", [128, 2, C]) for l in range(2)]
    in_RD = [inp(f"rd{l}", [C, C]) for l in range(2)]
    in_mu = [inp(f"mu{l}r", [128, K, D]) for l in range(2)]
    in_sg = [inp(f"sg{l}r", [128, K, D]) for l in range(2)]
    in_gm = [inp(f"gamma{l}", [C, 1]) for l in range(2)]
    in_bt = [inp(f"beta{l}", [C, 1]) for l in range(2)]
    out_h = nc.dram_tensor("out", [node_pad, C], f32, kind="ExternalOutput").ap()
    if DEBUG_TAPS:
        dbg_h0 = nc.dram_tensor("dbg_h0", [128, win, C], f32, kind="ExternalOutput").ap()
        dbg_hT0 = nc.dram_tensor("dbg_hT0", [C, node_pad], f32, kind="ExternalOutput").ap()
        dbg_tab1 = nc.dram_tensor("dbg_tab1", [trows, C], f32, kind="ExternalOutput").ap()
        dbg_st0 = nc.dram_tensor("dbg_st0", [C, 2], f32, kind="ExternalOutput").ap()

    with tile.TileContext(nc) as tc:
        nc.gpsimd.load_library(mlp)
        with tc.tile_pool(name="const", bufs=1) as cpool, \
             tc.tile_pool(name="sg", bufs=2) as sgp, \
             tc.tile_pool(name="wn", bufs=2) as wnp, \
             tc.tile_pool(name="per", bufs=1) as per, \
             tc.tile_pool(name="pB", bufs=2, space="PSUM") as pBp, \
             tc.tile_pool(name="pT", bufs=2, space="PSUM") as pTp, \
             tc.tile_pool(name="pH", bufs=2, space="PSUM") as pHp, \
             tc.tile_pool(name="pS", bufs=1, space="PSUM") as pSp, \
             tc.tile_pool(name="dram", bufs=1, space="DRAM") as dram:

            iota = cpool.tile([128, 128], f32)
            nc.sync.dma_start(iota[:], in_iota[:])
            ident = cpool.tile([128, 128], f32)
            nc.sync.dma_start(ident[:], in_ident[:])
            ones = cpool.tile([128, 2], f32)
            nc.sync.dma_start(ones[:], in_ones[:])
            ivd = cpool.tile([128, win], f32)
            nc.sync.dma_start(ivd[:], in_ivd[:])

            G_sb, RD_sb, gm_sb, bt_sb, quad = [], [], [], [], []
            for l in range(2):
                g_t = cpool.tile([128, 2, C], f32, tag=f"G{l}")
                nc.sync.dma_start(g_t[:], in_G[l][:])
                G_sb.append(g_t)
                rd_t = cpool.tile([C, C], f32, tag=f"RD{l}")
                nc.sync.dma_start(rd_t[:], in_RD[l][:])
                RD_sb.append(rd_t)
                gm_t = cpool.tile([C, 1], f32, tag=f"gm{l}")
                nc.sync.dma_start(gm_t[:], in_gm[l][:])
                gm_sb.append(gm_t)
                bt_t = cpool.tile([C, 1], f32, tag=f"bt{l}")
                nc.sync.dma_start(bt_t[:], in_bt[l][:])
                bt_sb.append(bt_t)
                mu_t = cpool.tile([128, K, D], f32, tag=f"mu{l}")
                nc.sync.dma_start(mu_t[:], in_mu[l][:])
                sg_t = cpool.tile([128, K, D], f32, tag=f"sg{l}")
                nc.sync.dma_start(sg_t[:], in_sg[l][:])
                # inv2s = 0.5 / (EPS + sigma^2), computed once on device
                s2 = cpool.tile([128, K, D], f32, tag=f"s2{l}")
                nc.vector.tensor_tensor(s2[:], sg_t[:], sg_t[:], OP.mult)
                nc.vector.tensor_scalar(s2[:], s2[:], EPS, None, OP.add)
                nc.vector.reciprocal(s2[:], s2[:])
                nc.vector.tensor_scalar(s2[:], s2[:], 0.5, None, OP.mult)
                quad.append((mu_t, s2))

            tab1 = dram.tile([trows, C], f32)
            ag_in = dram.tile([node_pad, C], f32)
            ar_in = [dram.tile([C, 2], f32, tag=f"ari{l}", name=f"ari{l}") for l in range(2)]
            ar_out = [dram.tile([C, 2], f32, tag=f"aro{l}", name=f"aro{l}") for l in range(2)]

            hT_prev = None
            for layer in range(2):
                tab_ap = in_tab if layer == 0 else tab1.opt()
                mu_t, inv_t = quad[layer]
                h_sb = per.tile([128, win, C], f32, tag="h", name=f"h{layer}")
                hTn = per.tile([C, node_pad], f32, tag=f"hT{layer}")
                pstat = pSp.tile([C, 1], f32, tag="st")
                pstat2 = pSp.tile([C, 1], f32, tag="st2")

                for sg_wins in sgs:
                    w0, wE = sg_wins[0], sg_wins[-1]
                    nlo = int(ncl[w0:wE + 1].sum())
                    nhi = int(nch[w0:wE + 1].sum())
                    nsg = nlo + nhi
                    clo0 = int(lo_off[w0])
                    chi0 = int(hi_off[w0])

                    xg = sgp.tile([128, nsg, C], f32, tag="xg")
                    idxs = sgp.tile([128, nsg * 8], i16, tag="idxs")
                    ps_t = sgp.tile([128, nsg, D], f32, tag="ps")
                    dw_t = sgp.tile([128, nsg], f32, tag="dw")

                    nc.sync.dma_start(idxs[:, :nlo * 8],
                                      in_idx[:, clo0 * 8:(clo0 + nlo) * 8])
                    nc.sync.dma_start(idxs[:, nlo * 8:],
                                      in_idx[:, chi0 * 8:(chi0 + nhi) * 8])
                    nc.sync.dma_start(ps_t[:, :nlo, :], in_ps[:, clo0:clo0 + nlo, :])
                    nc.sync.dma_start(ps_t[:, nlo:, :], in_ps[:, chi0:chi0 + nhi, :])
                    nc.sync.dma_start(dw_t[:, :nlo], in_dw[:, clo0:clo0 + nlo])
                    nc.sync.dma_start(dw_t[:, nlo:], in_dw[:, chi0:chi0 + nhi])

                    # split gathers at GMAX idxs: single_packet coalesces each
                    # engine's descs into ONE packet (<=64 descs/engine spec cap
                    # => <=1024 idxs per instruction; larger wedges the SDMA)
                    GMAX = 8  # chunks per gather instruction (8*128 = 1024 idx)
                    for (base, n, tview) in (
                            (0, nlo, tab_ap[0:HALF_SPLIT, :]),
                            (nlo, nhi, tab_ap[HALF_SPLIT:trows, :])):
                        for g0 in range(0, n, GMAX):
                            gn = min(GMAX, n - g0)
                            c0 = base + g0
                            nc.gpsimd.dma_gather(
                                xg[:, c0:c0 + gn, :], tview,
                                idxs[:, c0 * 8:(c0 + gn) * 8],
                                gn * 128, gn * 128, C)

                    # GMM weights: w[p, j, k] = exp(-sum_d inv2s*(ps - mu)^2)
                    dif = sgp.tile([128, nsg, K, D], f32, tag="dif")
                    nc.vector.tensor_tensor(
                        dif[:],
                        ps_t[:].unsqueeze(2).broadcast_to((128, nsg, K, D)),
                        mu_t[:].unsqueeze(1).broadcast_to((128, nsg, K, D)),
                        OP.subtract)
                    nc.vector.tensor_tensor(dif[:], dif[:], dif[:], OP.mult)
                    nc.vector.tensor_tensor(
                        dif[:], dif[:],
                        inv_t[:].unsqueeze(1).broadcast_to((128, nsg, K, D)),
                        OP.mult)
                    zt = sgp.tile([128, nsg, K], f32, tag="zt")
                    nc.vector.tensor_reduce(zt[:], dif[:], mybir.AxisListType.X, OP.add)
                    w_t = sgp.tile([128, nsg, K], f32, tag="wt")
                    nc.scalar.activation(w_t[:], zt[:], AF.Exp, scale=-1.0)

                    lo_c = 0
                    hi_c = nlo
                    for wi_ in sg_wins:
                        nl = int(ncl[wi_])
                        nh = int(nch[wi_])
                        ncw = nl + nh
                        ranges = [(lo_c, nl, 0), (hi_c, nh, nl)]

                        S = wnp.tile([128, ncw, 128], f32, tag="S")
                        xw = wnp.tile([128, ncw, K, C], f32, tag="xw")
                        for (c0, n, o) in ranges:
                            nc.vector.tensor_tensor(
                                S[:, o:o + n, :],
                                dw_t[:, c0:c0 + n].unsqueeze(2).broadcast_to((128, n, 128)),
                                iota[:].unsqueeze(1).broadcast_to((128, n, 128)),
                                OP.is_equal)
                            nc.vector.tensor_tensor(
                                xw[:, o:o + n, :, :],
                                xg[:, c0:c0 + n, :].unsqueeze(2).broadcast_to((128, n, K, C)),
                                w_t[:, c0:c0 + n, :].unsqueeze(3).broadcast_to((128, n, K, C)),
                                OP.mult)

                        pB = pBp.tile([128, K * C], f32, tag="pB")
                        for cj in range(ncw):
                            nc.tensor.matmul(
                                pB[:], S[:, cj, :], xw[:, cj, :, :].opt(),
                                start=(cj == 0), stop=(cj == ncw - 1))

                        bsb = wnp.tile([128, K * C], f32, tag="bsb")
                        nc.vector.tensor_scalar(
                            bsb[:], pB[:], ivd[:, wi_:wi_ + 1], None, OP.mult)

                        pT = pTp.tile([128, K * C], f32, tag="pT")
                        nc.tensor.transpose(pT[:, 0:128], bsb[:, 0:128], ident[:])
                        nc.tensor.transpose(pT[:, 128:256], bsb[:, 128:256], ident[:])
                        bT = wnp.tile([128, K * C], f32, tag="bT")
                        nc.vector.tensor_copy(bT[:], pT[:])

                        if layer == 0:
                            xTw = wnp.tile([C, 128], f32, tag="xTw")
                            nc.sync.dma_start(
                                xTw[:], in_xT[:, wi_ * 128:(wi_ + 1) * 128])
                            xT_ap = xTw[:]
                        else:
                            xT_ap = hT_prev[:, wi_ * 128:(wi_ + 1) * 128]

                        pH = pHp.tile([128, C], f32, tag="pH")
                        nc.tensor.matmul(pH[:], bT[:, 0:128], G_sb[layer][:, 0, :],
                                         start=True, stop=False)
                        nc.tensor.matmul(pH[:], bT[:, 128:256], G_sb[layer][:, 1, :],
                                         start=False, stop=False)
                        nc.tensor.matmul(pH[:], xT_ap, RD_sb[layer][:],
                                         start=False, stop=True)

                        nc.vector.tensor_copy(h_sb[:, wi_, :], pH[:])
                        hsq = wnp.tile([128, C], f32, tag="hsq")
                        nc.scalar.activation(hsq[:], h_sb[:, wi_, :], AF.Square)
                        mcol = 1 if wi_ == win - 1 else 0
                        nc.tensor.matmul(pstat[:], h_sb[:, wi_, :],
                                         ones[:, mcol:mcol + 1],
                                         start=(wi_ == 0), stop=(wi_ == win - 1),
                                         skip_group_check=True)
                        nc.tensor.matmul(pstat2[:], hsq[:],
                                         ones[:, mcol:mcol + 1],
                                         start=(wi_ == 0), stop=(wi_ == win - 1),
                                         skip_group_check=True)
                        lo_c += nl
                        hi_c += nh

                # BN stats all-reduce
                st = per.tile([C, 2], f32, tag=f"stsb{layer}")
                nc.vector.tensor_copy(st[:, 0:1], pstat[:])
                nc.vector.tensor_copy(st[:, 1:2], pstat2[:])
                nc.sync.dma_start(ar_in[layer][:], st[:])
                nc.gpsimd.collective_compute(
                    "AllReduce", OP.add,
                    replica_groups=[list(range(NCORES))],
                    ins=[ar_in[layer].opt()], outs=[ar_out[layer].opt()])
                stg = per.tile([C, 2], f32, tag=f"stg{layer}")
                nc.sync.dma_start(stg[:], ar_out[layer][:])

                mean = per.tile([C, 1], f32, tag=f"mean{layer}")
                nc.vector.tensor_scalar(mean[:], stg[:, 0:1], 1.0 / nfull, None, OP.mult)
                var = per.tile([C, 1], f32, tag=f"var{layer}")
                nc.vector.tensor_scalar(var[:], stg[:, 1:2], 1.0 / nfull, None, OP.mult)
                msq = per.tile([C, 1], f32, tag=f"msq{layer}")
                nc.vector.tensor_tensor(msq[:], mean[:], mean[:], OP.mult)
                nc.vector.tensor_tensor(var[:], var[:], msq[:], OP.subtract)
                nc.vector.tensor_scalar(var[:], var[:], BN_EPS, None, OP.add)
                sd = per.tile([C, 1], f32, tag=f"sd{layer}")
                nc.scalar.activation(sd[:], var[:], AF.Sqrt)
                rstd = per.tile([C, 1], f32, tag=f"rstd{layer}")
                nc.vector.reciprocal(rstd[:], sd[:])
                scl = per.tile([C, 1], f32, tag=f"scl{layer}")
                nc.vector.tensor_tensor(scl[:], gm_sb[layer][:], rstd[:], OP.mult)
                sh = per.tile([C, 1], f32, tag=f"sh{layer}")
                nc.vector.tensor_tensor(sh[:], mean[:], scl[:], OP.mult)
                nc.vector.tensor_tensor(sh[:], bt_sb[layer][:], sh[:], OP.subtract)

                # BN(+ReLU) in transposed domain
                bn_func = AF.Relu if layer == 0 else AF.Identity
                for wi_ in range(win):
                    pT2 = pTp.tile([C, 128], f32, tag="pT")
                    nc.tensor.transpose(pT2[:], h_sb[:, wi_, :], ident[:])
                    nc.scalar.activation(
                        hTn[:, wi_ * 128:(wi_ + 1) * 128], pT2[:],
                        bn_func, bias=sh[:], scale=scl[:])

                # transpose back to node-major
                hn = per.tile([128, win, C], f32, tag="hn", name=f"hn{layer}")
                for wi_ in range(win):
                    pN = pHp.tile([128, C], f32, tag="pH")
                    nc.tensor.matmul(pN[:], hTn[:, wi_ * 128:(wi_ + 1) * 128],
                                     ident[0:C, 0:C], is_transpose=True)
                    nc.vector.tensor_copy(hn[:, wi_, :], pN[:])

                if layer == 0:
                    ag_view = ag_in.opt().rearrange("(w p) c -> p w c", p=128)
                    nc.sync.dma_start(ag_view, hn[:])
                    nc.gpsimd.collective_compute(
                        "AllGather", OP.bypass,
                        replica_groups=[list(range(NCORES))],
                        ins=[ag_in.opt()], outs=[tab1.opt()])
                    hT_prev = hTn
                    if DEBUG_TAPS:
                        nc.sync.dma_start(dbg_h0[:], h_sb[:])
                        nc.sync.dma_start(dbg_hT0[:], hTn[:])
                        nc.sync.dma_start(dbg_tab1[:], tab1.opt())
                        nc.sync.dma_start(dbg_st0[:], stg[:])
                else:
                    out_view = out_h.rearrange("(w p) c -> p w c", p=128)
                    nc.sync.dma_start(out_view, hn[:])

    nc.compile()
    return nc


def make_in_maps(prep, inputs):
    npc, win, node_pad, trows = _derived()
    vals = np.asarray(inputs["vals"], np.float32)
    iota = np.broadcast_to(np.arange(128, dtype=np.float32), (128, 128)).copy()
    ident = np.eye(128, dtype=np.float32)

    tab = np.zeros((trows, C), np.float32)
    for m in range(NCORES):
        tab[m * node_pad:m * node_pad + npc] = vals[m * npc:(m + 1) * npc]

    ncht = prep["ncht"]
    onesv = np.zeros((128, 2), np.float32)
    onesv[:, 0] = 1.0
    tail = npc - (win - 1) * 128
    onesv[:tail, 1] = 1.0

    shared = {"iota": iota, "ident": ident, "onesv": onesv}
    for l in range(2):
        g = np.asarray(inputs[f"g{l}"], np.float32)          # [C, K*C]
        G = np.zeros((K * C, C), np.float32)                 # G[k*C+c, c'] = g[c, k*C+c']
        for k in range(K):
            G[k * C:(k + 1) * C, :] = g[:, k * C:(k + 1) * C]
        shared[f"g{l}c"] = G.reshape(2, 128, C).transpose(1, 0, 2).copy()
        shared[f"rd{l}"] = (np.asarray(inputs[f"root{l}"], np.float32)
                            + np.asarray(inputs[f"dense{l}"], np.float32))
        shared[f"mu{l}r"] = np.broadcast_to(
            np.asarray(inputs[f"mu{l}"], np.float32), (128, K, D)).copy()
        shared[f"sg{l}r"] = np.broadcast_to(
            np.asarray(inputs[f"sigma{l}"], np.float32), (128, K, D)).copy()
        shared[f"gamma{l}"] = np.asarray(inputs[f"gamma{l}"], np.float32).reshape(C, 1)
        shared[f"beta{l}"] = np.asarray(inputs[f"beta{l}"], np.float32).reshape(C, 1)

    in_maps = []
    for m in range(NCORES):
        nslots = ncht * 128
        blk = np.zeros((16, nslots // 16), np.int16)
        s = np.arange(nslots)
        blk[s % 16, s // 16] = prep["idx16"][m]
        idx_w = np.tile(blk, (8, 1))

        ps = np.zeros((128, ncht, D), np.float32)
        ps[s % 128, s // 128] = prep["pseudo_slot"][m]
        dw = np.full((128, ncht), -1.0, np.float32)
        dw[s % 128, s // 128] = prep["dstw"][m]

        xT0 = np.zeros((C, node_pad), np.float32)
        xT0[:, :npc] = vals[m * npc:(m + 1) * npc].T

        in_maps.append(dict(shared, tab=tab, idx=idx_w, pseudo=ps, dstw=dw,
                            xT0=xT0, invdeg=prep["invdeg"][m]))
    return in_maps


def kernel(**inputs):
    global LAST_RESULT
    from concourse.bass_utils import run_bass_kernel_spmd

    npc, win, node_pad, trows = _derived()
    prep = host_prep(np.asarray(inputs["edges"]))
    fill_pseudo(prep, inputs["pseudo"])
    nc = build_program(prep)
    in_maps = make_in_maps(prep, inputs)
    trace = bool(os.environ.get("BASS_KERNEL_TRACE"))
    import time as _time
    _t0 = _time.time()
    res = run_bass_kernel_spmd(nc, in_maps, list(range(NCORES)), trace=trace)
    print(f"[kernel] run_bass_kernel_spmd wall: {_time.time() - _t0:.2f}s", file=sys.stderr)
    LAST_RESULT = res
    out = np.concatenate(
        [res.results[m]["out"][:npc] for m in range(NCORES)], axis=0)
    return np.ascontiguousarray(out, dtype=np.float32)


# revision 15
# speedup vs baseline: 1.7655x; 1.7655x over previous
"""Trainium2 Bass kernel for nn_DenseReluGMMConvNetwork (2-layer GMMConv GNN).

Self-contained: takes FULL inputs, shards nodes across 8 NeuronCores,
runs one SPMD Bass program (gather / GMM weights / scatter-matmul /
BN via AllReduce / inter-layer AllGather), returns FULL [50000, 64] output.
"""

import os
import sys

sys.path.insert(0, "/opt/trn_rl_repo")

import numpy as np

# ---- problem constants (overridable for small-scale sim tests) ----
N = 50000
E = 800000
D = 3
K = 4
C = 64
NCORES = 8
EPS = 1e-15
BN_EPS = 1e-5
SG_WINDOWS = 4          # windows per gather super-group
HALF_SPLIT = 32768      # int16 index range split
DEBUG_TAPS = False

LAST_RESULT = None


def _derived():
    npc = N // NCORES
    win = (npc + 127) // 128          # windows per core
    node_pad = win * 128              # padded rows per core
    trows = NCORES * node_pad         # padded gather-table rows
    return npc, win, node_pad, trows


def host_prep(edges):
    """Route + sort edges, build the uniform chunk grid and slot arrays.

    Returns dict with per-core routed arrays and the static chunk grid.
    """
    npc, win, node_pad, trows = _derived()
    src = np.asarray(edges[0], np.int64)
    dst = np.asarray(edges[1], np.int64)
    e = src.shape[0]

    core = dst // npc
    dl = dst - core * npc
    wi = dl >> 7
    dj = dl & 127
    prow = (src // npc) * node_pad + (src % npc)
    half = (prow >= HALF_SPLIT).astype(np.int64)

    # group key: (core, half, wi); lo region first per core
    gkey = (core * 2 + half) * win + wi
    order = np.argsort(gkey, kind="stable")
    cnt = np.bincount(gkey, minlength=NCORES * 2 * win)
    goff = np.zeros_like(cnt)
    goff[1:] = np.cumsum(cnt)[:-1]
    rank = np.arange(e) - goff[gkey[order]]         # rank within group (sorted order)

    cnt3 = cnt.reshape(NCORES, 2, win)
    ncl = (cnt3[:, 0, :].max(axis=0) + 127) // 128  # lo chunks per window (shared)
    nch = (cnt3[:, 1, :].max(axis=0) + 127) // 128  # hi chunks per window
    ncl = np.maximum(ncl, 1)
    nch = np.maximum(nch, 1)
    nchl_tot = int(ncl.sum())
    nchh_tot = int(nch.sum())
    ncht = nchl_tot + nchh_tot
    lo_off = np.zeros(win, np.int64)
    lo_off[1:] = np.cumsum(ncl)[:-1]
    hi_off = np.zeros(win, np.int64)
    hi_off[1:] = np.cumsum(nch)[:-1]
    hi_off += nchl_tot

    # slot index for each (sorted) edge
    so = order
    chunk_base = np.where(half[so] == 0, lo_off[wi[so]], hi_off[wi[so]])
    slot = chunk_base * 128 + rank

    nslots = ncht * 128
    idx16 = np.zeros((NCORES, nslots), np.int16)
    pseudo_slot = np.zeros((NCORES, nslots, D), np.float32)
    dstw = np.full((NCORES, nslots), -1.0, np.float32)

    cs = core[so]
    iv = prow[so] - half[so] * HALF_SPLIT
    idx16[cs, slot] = iv.astype(np.int16)
    dstw[cs, slot] = dj[so].astype(np.float32)

    deg = np.bincount(dst, minlength=N).astype(np.float32)
    invdeg_flat = 1.0 / np.maximum(deg, 1.0)
    invdeg = np.zeros((NCORES, 128, win), np.float32)
    for m in range(NCORES):
        v = np.zeros(node_pad, np.float32)
        v[:npc] = invdeg_flat[m * npc:(m + 1) * npc]
        invdeg[m] = v.reshape(win, 128).T

    return dict(
        order=order, slot=slot, core_sorted=cs,
        ncl=ncl, nch=nch, ncht=ncht, nchl_tot=nchl_tot,
        lo_off=lo_off, hi_off=hi_off,
        idx16=idx16, pseudo_slot=pseudo_slot, dstw=dstw, invdeg=invdeg,
        npc=npc, win=win, node_pad=node_pad, trows=trows,
    )


def fill_pseudo(prep, pseudo):
    ps = np.asarray(pseudo, np.float32)
    prep["pseudo_slot"][:] = 0.0
    prep["pseudo_slot"][prep["core_sorted"], prep["slot"]] = ps[prep["order"]]


def build_program(prep):
    import concourse.bacc as bacc
    import concourse.mybir as mybir
    import concourse.tile as tile
    from concourse.library_config import mlp

    f32 = mybir.dt.float32
    i16 = mybir.dt.int16
    AF = mybir.ActivationFunctionType
    OP = mybir.AluOpType

    win = prep["win"]
    node_pad = prep["node_pad"]
    trows = prep["trows"]
    ncht = prep["ncht"]
    ncl = prep["ncl"]
    nch = prep["nch"]
    lo_off = prep["lo_off"]
    hi_off = prep["hi_off"]
    nfull = float(N)

    # super-groups of windows
    sgs = [list(range(s, min(s + SG_WINDOWS, win))) for s in range(0, win, SG_WINDOWS)]

    nc = bacc.Bacc("TRN2", target_bir_lowering=False, num_devices=NCORES,
                   num_swdge_queues=4)

    def inp(name, shape, dt=f32):
        return nc.dram_tensor(name, shape, dt, kind="ExternalInput").ap()

    in_tab = inp("tab", [trows, C])
    in_idx = inp("idx", [128, ncht * 8], i16)
    in_ps = inp("pseudo", [128, ncht, D])
    in_dw = inp("dstw", [128, ncht])
    in_xT = inp("xT0", [C, node_pad])
    in_ivd = inp("invdeg", [128, win])
    in_iota = inp("iota", [128, 128])
    in_ident = inp("ident", [128, 128])
    in_ones = inp("onesv", [128, 2])          # col0: ones, col1: valid mask last window
    in_G = [inp(f"g{l}c", [128, 2, C]) for l in range(2)]
    in_RD = [inp(f"rd{l}", [C, C]) for l in range(2)]
    in_mu = [inp(f"mu{l}r", [128, K, D]) for l in range(2)]
    in_sg = [inp(f"sg{l}r", [128, K, D]) for l in range(2)]
    in_gm = [inp(f"gamma{l}", [C, 1]) for l in range(2)]
    in_bt = [inp(f"beta{l}", [C, 1]) for l in range(2)]
    out_h = nc.dram_tensor("out", [node_pad, C], f32, kind="ExternalOutput").ap()
    if DEBUG_TAPS:
        dbg_h0 = nc.dram_tensor("dbg_h0", [128, win, C], f32, kind="ExternalOutput").ap()
        dbg_hT0 = nc.dram_tensor("dbg_hT0", [C, node_pad], f32, kind="ExternalOutput").ap()
        dbg_tab1 = nc.dram_tensor("dbg_tab1", [trows, C], f32, kind="ExternalOutput").ap()
        dbg_st0 = nc.dram_tensor("dbg_st0", [C, 2], f32, kind="ExternalOutput").ap()

    with tile.TileContext(nc) as tc:
        nc.gpsimd.load_library(mlp)
        with tc.tile_pool(name="const", bufs=1) as cpool, \
             tc.tile_pool(name="sg", bufs=2) as sgp, \
             tc.tile_pool(name="wn", bufs=2) as wnp, \
             tc.tile_pool(name="per", bufs=1) as per, \
             tc.tile_pool(name="pB", bufs=2, space="PSUM") as pBp, \
             tc.tile_pool(name="pT", bufs=2, space="PSUM") as pTp, \
             tc.tile_pool(name="pH", bufs=2, space="PSUM") as pHp, \
             tc.tile_pool(name="pS", bufs=1, space="PSUM") as pSp, \
             tc.tile_pool(name="dram", bufs=1, space="DRAM") as dram:

            iota = cpool.tile([128, 128], f32)
            nc.sync.dma_start(iota[:], in_iota[:])
            ident = cpool.tile([128, 128], f32)
            nc.sync.dma_start(ident[:], in_ident[:])
            ones = cpool.tile([128, 2], f32)
            nc.sync.dma_start(ones[:], in_ones[:])
            ivd = cpool.tile([128, win], f32)
            nc.sync.dma_start(ivd[:], in_ivd[:])

            G_sb, RD_sb, gm_sb, bt_sb, quad = [], [], [], [], []
            for l in range(2):
                g_t = cpool.tile([128, 2, C], f32, tag=f"G{l}")
                nc.sync.dma_start(g_t[:], in_G[l][:])
                G_sb.append(g_t)
                rd_t = cpool.tile([C, C], f32, tag=f"RD{l}")
                nc.sync.dma_start(rd_t[:], in_RD[l][:])
                RD_sb.append(rd_t)
                gm_t = cpool.tile([C, 1], f32, tag=f"gm{l}")
                nc.sync.dma_start(gm_t[:], in_gm[l][:])
                gm_sb.append(gm_t)
                bt_t = cpool.tile([C, 1], f32, tag=f"bt{l}")
                nc.sync.dma_start(bt_t[:], in_bt[l][:])
                bt_sb.append(bt_t)
                mu_t = cpool.tile([128, K, D], f32, tag=f"mu{l}")
                nc.sync.dma_start(mu_t[:], in_mu[l][:])
                sg_t = cpool.tile([128, K, D], f32, tag=f"sg{l}")
                nc.sync.dma_start(sg_t[:], in_sg[l][:])
                # inv2s = 0.5 / (EPS + sigma^2), computed once on device
                s2 = cpool.tile([128, K, D], f32, tag=f"s2{l}")
                nc.vector.tensor_tensor(s2[:], sg_t[:], sg_t[:], OP.mult)
                nc.vector.tensor_scalar(s2[:], s2[:], EPS, None, OP.add)
                nc.vector.reciprocal(s2[:], s2[:])
                nc.vector.tensor_scalar(s2[:], s2[:], 0.5, None, OP.mult)
                quad.append((mu_t, s2))

            tab1 = dram.tile([trows, C], f32)
            ag_in = dram.tile([node_pad, C], f32)
            ar_in = [dram.tile([C, 2], f32, tag=f"ari{l}", name=f"ari{l}") for l in range(2)]
            ar_out = [dram.tile([C, 2], f32, tag=f"aro{l}", name=f"aro{l}") for l in range(2)]

            hT_prev = None
            for layer in range(2):
                tab_ap = in_tab if layer == 0 else tab1.opt()
                mu_t, inv_t = quad[layer]
                h_sb = per.tile([128, win, C], f32, tag="h", name=f"h{layer}")
                hTn = per.tile([C, node_pad], f32, tag=f"hT{layer}")
                pstat = pSp.tile([C, 1], f32, tag="st")
                pstat2 = pSp.tile([C, 1], f32, tag="st2")

                gq = [0]
                for sg_wins in sgs:
                    w0, wE = sg_wins[0], sg_wins[-1]
                    nlo = int(ncl[w0:wE + 1].sum())
                    nhi = int(nch[w0:wE + 1].sum())
                    nsg = nlo + nhi
                    clo0 = int(lo_off[w0])
                    chi0 = int(hi_off[w0])

                    xg = sgp.tile([128, nsg, C], f32, tag="xg")
                    idxs = sgp.tile([128, nsg * 8], i16, tag="idxs")
                    ps_t = sgp.tile([128, nsg, D], f32, tag="ps")
                    dw_t = sgp.tile([128, nsg], f32, tag="dw")

                    nc.sync.dma_start(idxs[:, :nlo * 8],
                                      in_idx[:, clo0 * 8:(clo0 + nlo) * 8])
                    nc.sync.dma_start(idxs[:, nlo * 8:],
                                      in_idx[:, chi0 * 8:(chi0 + nhi) * 8])
                    nc.sync.dma_start(ps_t[:, :nlo, :], in_ps[:, clo0:clo0 + nlo, :])
                    nc.sync.dma_start(ps_t[:, nlo:, :], in_ps[:, chi0:chi0 + nhi, :])
                    nc.sync.dma_start(dw_t[:, :nlo], in_dw[:, clo0:clo0 + nlo])
                    nc.sync.dma_start(dw_t[:, nlo:], in_dw[:, chi0:chi0 + nhi])

                    # split gathers at GMAX idxs: single_packet coalesces each
                    # engine's descs into ONE packet (<=64 descs/engine spec cap
                    # => <=1024 idxs per instruction; larger wedges the SDMA)
                    GMAX = 8  # chunks per gather instruction (8*128 = 1024 idx)
                    for (base, n, tview) in (
                            (0, nlo, tab_ap[0:HALF_SPLIT, :]),
                            (nlo, nhi, tab_ap[HALF_SPLIT:trows, :])):
                        for g0 in range(0, n, GMAX):
                            gn = min(GMAX, n - g0)
                            c0 = base + g0
                            nc.gpsimd.dma_gather(
                                xg[:, c0:c0 + gn, :], tview,
                                idxs[:, c0 * 8:(c0 + gn) * 8],
                                gn * 128, gn * 128, C,
                                queue_num=gq[0] % 4)
                            gq[0] += 1

                    # GMM weights: w[p, j, k] = exp(-sum_d inv2s*(ps - mu)^2)
                    dif = sgp.tile([128, nsg, K, D], f32, tag="dif")
                    nc.vector.tensor_tensor(
                        dif[:],
                        ps_t[:].unsqueeze(2).broadcast_to((128, nsg, K, D)),
                        mu_t[:].unsqueeze(1).broadcast_to((128, nsg, K, D)),
                        OP.subtract)
                    nc.vector.tensor_tensor(dif[:], dif[:], dif[:], OP.mult)
                    nc.vector.tensor_tensor(
                        dif[:], dif[:],
                        inv_t[:].unsqueeze(1).broadcast_to((128, nsg, K, D)),
                        OP.mult)
                    zt = sgp.tile([128, nsg, K], f32, tag="zt")
                    nc.vector.tensor_reduce(zt[:], dif[:], mybir.AxisListType.X, OP.add)
                    w_t = sgp.tile([128, nsg, K], f32, tag="wt")
                    nc.scalar.activation(w_t[:], zt[:], AF.Exp, scale=-1.0)

                    lo_c = 0
                    hi_c = nlo
                    for wi_ in sg_wins:
                        nl = int(ncl[wi_])
                        nh = int(nch[wi_])
                        ncw = nl + nh
                        ranges = [(lo_c, nl, 0), (hi_c, nh, nl)]

                        S = wnp.tile([128, ncw, 128], f32, tag="S")
                        xw = wnp.tile([128, ncw, K, C], f32, tag="xw")
                        for (c0, n, o) in ranges:
                            nc.vector.tensor_tensor(
                                S[:, o:o + n, :],
                                dw_t[:, c0:c0 + n].unsqueeze(2).broadcast_to((128, n, 128)),
                                iota[:].unsqueeze(1).broadcast_to((128, n, 128)),
                                OP.is_equal)
                            nc.vector.tensor_tensor(
                                xw[:, o:o + n, :, :],
                                xg[:, c0:c0 + n, :].unsqueeze(2).broadcast_to((128, n, K, C)),
                                w_t[:, c0:c0 + n, :].unsqueeze(3).broadcast_to((128, n, K, C)),
                                OP.mult)

                        pB = pBp.tile([128, K * C], f32, tag="pB")
                        for cj in range(ncw):
                            nc.tensor.matmul(
                                pB[:], S[:, cj, :], xw[:, cj, :, :].opt(),
                                start=(cj == 0), stop=(cj == ncw - 1))

                        bsb = wnp.tile([128, K * C], f32, tag="bsb")
                        nc.vector.tensor_scalar(
                            bsb[:], pB[:], ivd[:, wi_:wi_ + 1], None, OP.mult)

                        pT = pTp.tile([128, K * C], f32, tag="pT")
                        nc.tensor.transpose(pT[:, 0:128], bsb[:, 0:128], ident[:])
                        nc.tensor.transpose(pT[:, 128:256], bsb[:, 128:256], ident[:])
                        bT = wnp.tile([128, K * C], f32, tag="bT")
                        nc.vector.tensor_copy(bT[:], pT[:])

                        if layer == 0:
                            xTw = wnp.tile([C, 128], f32, tag="xTw")
                            nc.sync.dma_start(
                                xTw[:], in_xT[:, wi_ * 128:(wi_ + 1) * 128])
                            xT_ap = xTw[:]
                        else:
                            xT_ap = hT_prev[:, wi_ * 128:(wi_ + 1) * 128]

                        pH = pHp.tile([128, C], f32, tag="pH")
                        nc.tensor.matmul(pH[:], bT[:, 0:128], G_sb[layer][:, 0, :],
                                         start=True, stop=False)
                        nc.tensor.matmul(pH[:], bT[:, 128:256], G_sb[layer][:, 1, :],
                                         start=False, stop=False)
                        nc.tensor.matmul(pH[:], xT_ap, RD_sb[layer][:],
                                         start=False, stop=True)

                        nc.vector.tensor_copy(h_sb[:, wi_, :], pH[:])
                        hsq = wnp.tile([128, C], f32, tag="hsq")
                        nc.scalar.activation(hsq[:], h_sb[:, wi_, :], AF.Square)
                        mcol = 1 if wi_ == win - 1 else 0
                        nc.tensor.matmul(pstat[:], h_sb[:, wi_, :],
                                         ones[:, mcol:mcol + 1],
                                         start=(wi_ == 0), stop=(wi_ == win - 1),
                                         skip_group_check=True)
                        nc.tensor.matmul(pstat2[:], hsq[:],
                                         ones[:, mcol:mcol + 1],
                                         start=(wi_ == 0), stop=(wi_ == win - 1),
                                         skip_group_check=True)
                        lo_c += nl
                        hi_c += nh

                # BN stats all-reduce
                st = per.tile([C, 2], f32, tag=f"stsb{layer}")
                nc.vector.tensor_copy(st[:, 0:1], pstat[:])
                nc.vector.tensor_copy(st[:, 1:2], pstat2[:])
                nc.sync.dma_start(ar_in[layer][:], st[:])
                nc.gpsimd.collective_compute(
                    "AllReduce", OP.add,
                    replica_groups=[list(range(NCORES))],
                    ins=[ar_in[layer].opt()], outs=[ar_out[layer].opt()])
                stg = per.tile([C, 2], f32, tag=f"stg{layer}")
                nc.sync.dma_start(stg[:], ar_out[layer][:])

                mean = per.tile([C, 1], f32, tag=f"mean{layer}")
                nc.vector.tensor_scalar(mean[:], stg[:, 0:1], 1.0 / nfull, None, OP.mult)
                var = per.tile([C, 1], f32, tag=f"var{layer}")
                nc.vector.tensor_scalar(var[:], stg[:, 1:2], 1.0 / nfull, None, OP.mult)
                msq = per.tile([C, 1], f32, tag=f"msq{layer}")
                nc.vector.tensor_tensor(msq[:], mean[:], mean[:], OP.mult)
                nc.vector.tensor_tensor(var[:], var[:], msq[:], OP.subtract)
                nc.vector.tensor_scalar(var[:], var[:], BN_EPS, None, OP.add)
                sd = per.tile([C, 1], f32, tag=f"sd{layer}")
                nc.scalar.activation(sd[:], var[:], AF.Sqrt)
                rstd = per.tile([C, 1], f32, tag=f"rstd{layer}")
                nc.vector.reciprocal(rstd[:], sd[:])
                scl = per.tile([C, 1], f32, tag=f"scl{layer}")
                nc.vector.tensor_tensor(scl[:], gm_sb[layer][:], rstd[:], OP.mult)
                sh = per.tile([C, 1], f32, tag=f"sh{layer}")
                nc.vector.tensor_tensor(sh[:], mean[:], scl[:], OP.mult)
                nc.vector.tensor_tensor(sh[:], bt_sb[layer][:], sh[:], OP.subtract)

                # BN(+ReLU) in transposed domain
                bn_func = AF.Relu if layer == 0 else AF.Identity
                for wi_ in range(win):
                    pT2 = pTp.tile([C, 128], f32, tag="pT")
                    nc.tensor.transpose(pT2[:], h_sb[:, wi_, :], ident[:])
                    nc.scalar.activation(
                        hTn[:, wi_ * 128:(wi_ + 1) * 128], pT2[:],
                        bn_func, bias=sh[:], scale=scl[:])

                # transpose back to node-major
                hn = per.tile([128, win, C], f32, tag="hn", name=f"hn{layer}")
                for wi_ in range(win):
                    pN = pHp.tile([128, C], f32, tag="pH")
                    nc.tensor.matmul(pN[:], hTn[:, wi_ * 128:(wi_ + 1) * 128],
                                     ident[0:C, 0:C], is_transpose=True)
                    nc.vector.tensor_copy(hn[:, wi_, :], pN[:])

                if layer == 0:
                    ag_view = ag_in.opt().rearrange("(w p) c -> p w c", p=128)
                    nc.sync.dma_start(ag_view, hn[:])
                    nc.gpsimd.collective_compute(
                        "AllGather", OP.bypass,
                        replica_groups=[list(range(NCORES))],
                        ins=[ag_in.opt()], outs=[tab1.opt()])
                    hT_prev = hTn
                    if DEBUG_TAPS:
                        nc.sync.dma_start(dbg_h0[:], h_sb[:])
                        nc.sync.dma_start(dbg_hT0[:], hTn[:])
                        nc.sync.dma_start(dbg_tab1[:], tab1.opt())
                        nc.sync.dma_start(dbg_st0[:], stg[:])
                else:
                    out_view = out_h.rearrange("(w p) c -> p w c", p=128)
                    nc.sync.dma_start(out_view, hn[:])

    nc.compile()
    return nc


def make_in_maps(prep, inputs):
    npc, win, node_pad, trows = _derived()
    vals = np.asarray(inputs["vals"], np.float32)
    iota = np.broadcast_to(np.arange(128, dtype=np.float32), (128, 128)).copy()
    ident = np.eye(128, dtype=np.float32)

    tab = np.zeros((trows, C), np.float32)
    for m in range(NCORES):
        tab[m * node_pad:m * node_pad + npc] = vals[m * npc:(m + 1) * npc]

    ncht = prep["ncht"]
    onesv = np.zeros((128, 2), np.float32)
    onesv[:, 0] = 1.0
    tail = npc - (win - 1) * 128
    onesv[:tail, 1] = 1.0

    shared = {"iota": iota, "ident": ident, "onesv": onesv}
    for l in range(2):
        g = np.asarray(inputs[f"g{l}"], np.float32)          # [C, K*C]
        G = np.zeros((K * C, C), np.float32)                 # G[k*C+c, c'] = g[c, k*C+c']
        for k in range(K):
            G[k * C:(k + 1) * C, :] = g[:, k * C:(k + 1) * C]
        shared[f"g{l}c"] = G.reshape(2, 128, C).transpose(1, 0, 2).copy()
        shared[f"rd{l}"] = (np.asarray(inputs[f"root{l}"], np.float32)
                            + np.asarray(inputs[f"dense{l}"], np.float32))
        shared[f"mu{l}r"] = np.broadcast_to(
            np.asarray(inputs[f"mu{l}"], np.float32), (128, K, D)).copy()
        shared[f"sg{l}r"] = np.broadcast_to(
            np.asarray(inputs[f"sigma{l}"], np.float32), (128, K, D)).copy()
        shared[f"gamma{l}"] = np.asarray(inputs[f"gamma{l}"], np.float32).reshape(C, 1)
        shared[f"beta{l}"] = np.asarray(inputs[f"beta{l}"], np.float32).reshape(C, 1)

    in_maps = []
    for m in range(NCORES):
        nslots = ncht * 128
        blk = np.zeros((16, nslots // 16), np.int16)
        s = np.arange(nslots)
        blk[s % 16, s // 16] = prep["idx16"][m]
        idx_w = np.tile(blk, (8, 1))

        ps = np.zeros((128, ncht, D), np.float32)
        ps[s % 128, s // 128] = prep["pseudo_slot"][m]
        dw = np.full((128, ncht), -1.0, np.float32)
        dw[s % 128, s // 128] = prep["dstw"][m]

        xT0 = np.zeros((C, node_pad), np.float32)
        xT0[:, :npc] = vals[m * npc:(m + 1) * npc].T

        in_maps.append(dict(shared, tab=tab, idx=idx_w, pseudo=ps, dstw=dw,
                            xT0=xT0, invdeg=prep["invdeg"][m]))
    return in_maps


def kernel(**inputs):
    global LAST_RESULT
    from concourse.bass_utils import run_bass_kernel_spmd

    npc, win, node_pad, trows = _derived()
    prep = host_prep(np.asarray(inputs["edges"]))
    fill_pseudo(prep, inputs["pseudo"])
    nc = build_program(prep)
    in_maps = make_in_maps(prep, inputs)
    trace = bool(os.environ.get("BASS_KERNEL_TRACE"))
    import time as _time
    _t0 = _time.time()
    res = run_bass_kernel_spmd(nc, in_maps, list(range(NCORES)), trace=trace)
    print(f"[kernel] run_bass_kernel_spmd wall: {_time.time() - _t0:.2f}s", file=sys.stderr)
    LAST_RESULT = res
    out = np.concatenate(
        [res.results[m]["out"][:npc] for m in range(NCORES)], axis=0)
    return np.ascontiguousarray(out, dtype=np.float32)


# revision 17
# speedup vs baseline: 1.7875x; 1.0125x over previous
"""Trainium2 Bass kernel for nn_DenseReluGMMConvNetwork (2-layer GMMConv GNN).

Self-contained: takes FULL inputs, shards nodes across 8 NeuronCores,
runs one SPMD Bass program (gather / GMM weights / scatter-matmul /
BN via AllReduce / inter-layer AllGather), returns FULL [50000, 64] output.
"""

import os
import sys

sys.path.insert(0, "/opt/trn_rl_repo")

import numpy as np

# ---- problem constants (overridable for small-scale sim tests) ----
N = 50000
E = 800000
D = 3
K = 4
C = 64
NCORES = 8
EPS = 1e-15
BN_EPS = 1e-5
SG_WINDOWS = 4          # windows per gather super-group
HALF_SPLIT = 32768      # int16 index range split
DEBUG_TAPS = False

LAST_RESULT = None


def _derived():
    npc = N // NCORES
    win = (npc + 127) // 128          # windows per core
    node_pad = win * 128              # padded rows per core
    trows = NCORES * node_pad         # padded gather-table rows
    return npc, win, node_pad, trows


def host_prep(edges):
    """Route + sort edges, build the uniform chunk grid and slot arrays.

    Returns dict with per-core routed arrays and the static chunk grid.
    """
    npc, win, node_pad, trows = _derived()
    src = np.asarray(edges[0], np.int64)
    dst = np.asarray(edges[1], np.int64)
    e = src.shape[0]

    core = dst // npc
    dl = dst - core * npc
    wi = dl >> 7
    dj = dl & 127
    prow = (src // npc) * node_pad + (src % npc)
    half = (prow >= HALF_SPLIT).astype(np.int64)

    # group key: (core, half, wi); lo region first per core
    gkey = (core * 2 + half) * win + wi
    order = np.argsort(gkey, kind="stable")
    cnt = np.bincount(gkey, minlength=NCORES * 2 * win)
    goff = np.zeros_like(cnt)
    goff[1:] = np.cumsum(cnt)[:-1]
    rank = np.arange(e) - goff[gkey[order]]         # rank within group (sorted order)

    cnt3 = cnt.reshape(NCORES, 2, win)
    ncl = (cnt3[:, 0, :].max(axis=0) + 127) // 128  # lo chunks per window (shared)
    nch = (cnt3[:, 1, :].max(axis=0) + 127) // 128  # hi chunks per window
    ncl = np.maximum(ncl, 1)
    nch = np.maximum(nch, 1)
    nchl_tot = int(ncl.sum())
    nchh_tot = int(nch.sum())
    ncht = nchl_tot + nchh_tot
    lo_off = np.zeros(win, np.int64)
    lo_off[1:] = np.cumsum(ncl)[:-1]
    hi_off = np.zeros(win, np.int64)
    hi_off[1:] = np.cumsum(nch)[:-1]
    hi_off += nchl_tot

    # slot index for each (sorted) edge
    so = order
    chunk_base = np.where(half[so] == 0, lo_off[wi[so]], hi_off[wi[so]])
    slot = chunk_base * 128 + rank

    nslots = ncht * 128
    idx16 = np.zeros((NCORES, nslots), np.int16)
    pseudo_slot = np.zeros((NCORES, nslots, D), np.float32)
    dstw = np.full((NCORES, nslots), -1.0, np.float32)

    cs = core[so]
    iv = prow[so] - half[so] * HALF_SPLIT
    idx16[cs, slot] = iv.astype(np.int16)
    dstw[cs, slot] = dj[so].astype(np.float32)

    deg = np.bincount(dst, minlength=N).astype(np.float32)
    invdeg_flat = 1.0 / np.maximum(deg, 1.0)
    invdeg = np.zeros((NCORES, 128, win), np.float32)
    for m in range(NCORES):
        v = np.zeros(node_pad, np.float32)
        v[:npc] = invdeg_flat[m * npc:(m + 1) * npc]
        invdeg[m] = v.reshape(win, 128).T

    return dict(
        order=order, slot=slot, core_sorted=cs,
        ncl=ncl, nch=nch, ncht=ncht, nchl_tot=nchl_tot,
        lo_off=lo_off, hi_off=hi_off,
        idx16=idx16, pseudo_slot=pseudo_slot, dstw=dstw, invdeg=invdeg,
        npc=npc, win=win, node_pad=node_pad, trows=trows,
    )


def fill_pseudo(prep, pseudo):
    ps = np.asarray(pseudo, np.float32)
    prep["pseudo_slot"][:] = 0.0
    prep["pseudo_slot"][prep["core_sorted"], prep["slot"]] = ps[prep["order"]]


def build_program(prep):
    import concourse.bacc as bacc
    import concourse.mybir as mybir
    import concourse.tile as tile
    from concourse.library_config import mlp

    f32 = mybir.dt.float32
    i16 = mybir.dt.int16
    AF = mybir.ActivationFunctionType
    OP = mybir.AluOpType

    win = prep["win"]
    node_pad = prep["node_pad"]
    trows = prep["trows"]
    ncht = prep["ncht"]
    ncl = prep["ncl"]
    nch = prep["nch"]
    lo_off = prep["lo_off"]
    hi_off = prep["hi_off"]
    nfull = float(N)

    # super-groups of windows
    sgs = [list(range(s, min(s + SG_WINDOWS, win))) for s in range(0, win, SG_WINDOWS)]

    nc = bacc.Bacc("TRN2", target_bir_lowering=False, num_devices=NCORES,
                   num_swdge_queues=4)

    def inp(name, shape, dt=f32):
        return nc.dram_tensor(name, shape, dt, kind="ExternalInput").ap()

    in_tab = inp("tab", [trows, C])
    in_idx = inp("idx", [128, ncht * 8], i16)
    in_ps = inp("pseudo", [128, ncht, D])
    in_dw = inp("dstw", [128, ncht])
    in_xT = inp("xT0", [C, node_pad])
    in_ivd = inp("invdeg", [128, win])
    in_iota = inp("iota", [128, 128])
    in_ident = inp("ident", [128, 128])
    in_ones = inp("onesv", [128, 2])          # col0: ones, col1: valid mask last window
    in_G = [inp(f"g{l}c", [128, 2, C]) for l in range(2)]
    in_RD = [inp(f"rd{l}", [C, C]) for l in range(2)]
    in_mu = [inp(f"mu{l}r", [128, K, D]) for l in range(2)]
    in_sg = [inp(f"sg{l}r", [128, K, D]) for l in range(2)]
    in_gm = [inp(f"gamma{l}", [C, 1]) for l in range(2)]
    in_bt = [inp(f"beta{l}", [C, 1]) for l in range(2)]
    out_h = nc.dram_tensor("out", [node_pad, C], f32, kind="ExternalOutput").ap()
    if DEBUG_TAPS:
        dbg_h0 = nc.dram_tensor("dbg_h0", [128, win, C], f32, kind="ExternalOutput").ap()
        dbg_hT0 = nc.dram_tensor("dbg_hT0", [C, node_pad], f32, kind="ExternalOutput").ap()
        dbg_tab1 = nc.dram_tensor("dbg_tab1", [trows, C], f32, kind="ExternalOutput").ap()
        dbg_st0 = nc.dram_tensor("dbg_st0", [C, 2], f32, kind="ExternalOutput").ap()

    with tile.TileContext(nc) as tc:
        nc.gpsimd.load_library(mlp)
        with tc.tile_pool(name="const", bufs=1) as cpool, \
             tc.tile_pool(name="sg", bufs=2) as sgp, \
             tc.tile_pool(name="wn", bufs=2) as wnp, \
             tc.tile_pool(name="per", bufs=1) as per, \
             tc.tile_pool(name="pB", bufs=2, space="PSUM") as pBp, \
             tc.tile_pool(name="pT", bufs=2, space="PSUM") as pTp, \
             tc.tile_pool(name="pH", bufs=2, space="PSUM") as pHp, \
             tc.tile_pool(name="pS", bufs=1, space="PSUM") as pSp, \
             tc.tile_pool(name="dram", bufs=1, space="DRAM") as dram:

            iota = cpool.tile([128, 128], f32)
            nc.sync.dma_start(iota[:], in_iota[:])
            ident = cpool.tile([128, 128], f32)
            nc.sync.dma_start(ident[:], in_ident[:])
            ones = cpool.tile([128, 2], f32)
            nc.sync.dma_start(ones[:], in_ones[:])
            ivd = cpool.tile([128, win], f32)
            nc.sync.dma_start(ivd[:], in_ivd[:])

            G_sb, RD_sb, gm_sb, bt_sb, quad = [], [], [], [], []
            for l in range(2):
                g_t = cpool.tile([128, 2, C], f32, tag=f"G{l}")
                nc.sync.dma_start(g_t[:], in_G[l][:])
                G_sb.append(g_t)
                rd_t = cpool.tile([C, C], f32, tag=f"RD{l}")
                nc.sync.dma_start(rd_t[:], in_RD[l][:])
                RD_sb.append(rd_t)
                gm_t = cpool.tile([C, 1], f32, tag=f"gm{l}")
                nc.sync.dma_start(gm_t[:], in_gm[l][:])
                gm_sb.append(gm_t)
                bt_t = cpool.tile([C, 1], f32, tag=f"bt{l}")
                nc.sync.dma_start(bt_t[:], in_bt[l][:])
                bt_sb.append(bt_t)
                mu_t = cpool.tile([128, K, D], f32, tag=f"mu{l}")
                nc.sync.dma_start(mu_t[:], in_mu[l][:])
                sg_t = cpool.tile([128, K, D], f32, tag=f"sg{l}")
                nc.sync.dma_start(sg_t[:], in_sg[l][:])
                # inv2s = 0.5 / (EPS + sigma^2), computed once on device
                s2 = cpool.tile([128, K, D], f32, tag=f"s2{l}")
                nc.vector.tensor_tensor(s2[:], sg_t[:], sg_t[:], OP.mult)
                nc.vector.tensor_scalar(s2[:], s2[:], EPS, None, OP.add)
                nc.vector.reciprocal(s2[:], s2[:])
                nc.vector.tensor_scalar(s2[:], s2[:], 0.5, None, OP.mult)
                quad.append((mu_t, s2))

            tab1 = dram.tile([trows, C], f32)
            ag_in = dram.tile([node_pad, C], f32)
            ar_in = [dram.tile([C, 2], f32, tag=f"ari{l}", name=f"ari{l}") for l in range(2)]
            ar_out = [dram.tile([C, 2], f32, tag=f"aro{l}", name=f"aro{l}") for l in range(2)]

            hT_prev = None
            for layer in range(2):
                tab_ap = in_tab if layer == 0 else tab1.opt()
                mu_t, inv_t = quad[layer]
                h_sb = per.tile([128, win, C], f32, tag="h", name=f"h{layer}")
                hTn = per.tile([C, node_pad], f32, tag=f"hT{layer}")
                pstat = pSp.tile([C, 1], f32, tag="st")
                pstat2 = pSp.tile([C, 1], f32, tag="st2")

                gq = [0]
                for sg_wins in sgs:
                    w0, wE = sg_wins[0], sg_wins[-1]
                    nlo = int(ncl[w0:wE + 1].sum())
                    nhi = int(nch[w0:wE + 1].sum())
                    nsg = nlo + nhi
                    clo0 = int(lo_off[w0])
                    chi0 = int(hi_off[w0])

                    xg = sgp.tile([128, nsg, C], f32, tag="xg")
                    idxs = sgp.tile([128, nsg * 8], i16, tag="idxs")
                    ps_t = sgp.tile([128, nsg, D], f32, tag="ps")
                    dw_t = sgp.tile([128, nsg], f32, tag="dw")

                    nc.sync.dma_start(idxs[:, :nlo * 8],
                                      in_idx[:, clo0 * 8:(clo0 + nlo) * 8])
                    nc.sync.dma_start(idxs[:, nlo * 8:],
                                      in_idx[:, chi0 * 8:(chi0 + nhi) * 8])
                    nc.sync.dma_start(ps_t[:, :nlo, :], in_ps[:, clo0:clo0 + nlo, :])
                    nc.sync.dma_start(ps_t[:, nlo:, :], in_ps[:, chi0:chi0 + nhi, :])
                    nc.sync.dma_start(dw_t[:, :nlo], in_dw[:, clo0:clo0 + nlo])
                    nc.sync.dma_start(dw_t[:, nlo:], in_dw[:, chi0:chi0 + nhi])

                    # split gathers at GMAX idxs: single_packet coalesces each
                    # engine's descs into ONE packet (<=64 descs/engine spec cap
                    # => <=1024 idxs per instruction; larger wedges the SDMA)
                    GMAX = 8  # chunks per gather instruction (8*128 = 1024 idx)
                    for (base, n, tview) in (
                            (0, nlo, tab_ap[0:HALF_SPLIT, :]),
                            (nlo, nhi, tab_ap[HALF_SPLIT:trows, :])):
                        for g0 in range(0, n, GMAX):
                            gn = min(GMAX, n - g0)
                            c0 = base + g0
                            nc.gpsimd.dma_gather(
                                xg[:, c0:c0 + gn, :], tview,
                                idxs[:, c0 * 8:(c0 + gn) * 8],
                                gn * 128, gn * 128, C,
                                queue_num=gq[0] % 4)
                            gq[0] += 1

                    # GMM weights: w[p, j, k] = exp(-sum_d inv2s*(ps - mu)^2)
                    dif = sgp.tile([128, nsg, K, D], f32, tag="dif")
                    nc.vector.tensor_tensor(
                        dif[:],
                        ps_t[:].unsqueeze(2).broadcast_to((128, nsg, K, D)),
                        mu_t[:].unsqueeze(1).broadcast_to((128, nsg, K, D)),
                        OP.subtract)
                    nc.vector.tensor_tensor(dif[:], dif[:], dif[:], OP.mult)
                    nc.vector.tensor_tensor(
                        dif[:], dif[:],
                        inv_t[:].unsqueeze(1).broadcast_to((128, nsg, K, D)),
                        OP.mult)
                    zt = sgp.tile([128, nsg, K], f32, tag="zt")
                    nc.vector.tensor_reduce(zt[:], dif[:], mybir.AxisListType.X, OP.add)
                    w_t = sgp.tile([128, nsg, K], f32, tag="wt")
                    nc.scalar.activation(w_t[:], zt[:], AF.Exp, scale=-1.0)

                    lo_c = 0
                    hi_c = nlo
                    for wi_ in sg_wins:
                        nl = int(ncl[wi_])
                        nh = int(nch[wi_])
                        ncw = nl + nh
                        ranges = [(lo_c, nl, 0), (hi_c, nh, nl)]

                        S = wnp.tile([128, ncw, 128], f32, tag="S")
                        xw = wnp.tile([128, ncw, K, C], f32, tag="xw")
                        for (c0, n, o) in ranges:
                            nc.vector.tensor_tensor(
                                S[:, o:o + n, :],
                                dw_t[:, c0:c0 + n].unsqueeze(2).broadcast_to((128, n, 128)),
                                iota[:].unsqueeze(1).broadcast_to((128, n, 128)),
                                OP.is_equal)
                            # xw split: k=0..2 on VectorE, k=3 on ScalarE
                            # (per-partition scale) to unblock the DVE-bound
                            # per-window pipeline
                            nc.vector.tensor_tensor(
                                xw[:, o:o + n, 0:3, :],
                                xg[:, c0:c0 + n, :].unsqueeze(2).broadcast_to((128, n, 3, C)),
                                w_t[:, c0:c0 + n, 0:3].unsqueeze(3).broadcast_to((128, n, 3, C)),
                                OP.mult)
                            for j in range(n):
                                nc.scalar.activation(
                                    xw[:, o + j, 3, :], xg[:, c0 + j, :],
                                    AF.Copy, scale=w_t[:, c0 + j, 3:4])

                        pB = pBp.tile([128, K * C], f32, tag="pB")
                        for cj in range(ncw):
                            nc.tensor.matmul(
                                pB[:], S[:, cj, :], xw[:, cj, :, :].opt(),
                                start=(cj == 0), stop=(cj == ncw - 1))

                        bsb = wnp.tile([128, K * C], f32, tag="bsb")
                        nc.vector.tensor_scalar(
                            bsb[:], pB[:], ivd[:, wi_:wi_ + 1], None, OP.mult)

                        pT = pTp.tile([128, K * C], f32, tag="pT")
                        nc.tensor.transpose(pT[:, 0:128], bsb[:, 0:128], ident[:])
                        nc.tensor.transpose(pT[:, 128:256], bsb[:, 128:256], ident[:])
                        bT = wnp.tile([128, K * C], f32, tag="bT")
                        nc.vector.tensor_copy(bT[:], pT[:])

                        if layer == 0:
                            xTw = wnp.tile([C, 128], f32, tag="xTw")
                            nc.sync.dma_start(
                                xTw[:], in_xT[:, wi_ * 128:(wi_ + 1) * 128])
                            xT_ap = xTw[:]
                        else:
                            xT_ap = hT_prev[:, wi_ * 128:(wi_ + 1) * 128]

                        pH = pHp.tile([128, C], f32, tag="pH")
                        nc.tensor.matmul(pH[:], bT[:, 0:128], G_sb[layer][:, 0, :],
                                         start=True, stop=False)
                        nc.tensor.matmul(pH[:], bT[:, 128:256], G_sb[layer][:, 1, :],
                                         start=False, stop=False)
                        nc.tensor.matmul(pH[:], xT_ap, RD_sb[layer][:],
                                         start=False, stop=True)

                        nc.vector.tensor_copy(h_sb[:, wi_, :], pH[:])
                        hsq = wnp.tile([128, C], f32, tag="hsq")
                        nc.scalar.activation(hsq[:], h_sb[:, wi_, :], AF.Square)
                        mcol = 1 if wi_ == win - 1 else 0
                        nc.tensor.matmul(pstat[:], h_sb[:, wi_, :],
                                         ones[:, mcol:mcol + 1],
                                         start=(wi_ == 0), stop=(wi_ == win - 1),
                                         skip_group_check=True)
                        nc.tensor.matmul(pstat2[:], hsq[:],
                                         ones[:, mcol:mcol + 1],
                                         start=(wi_ == 0), stop=(wi_ == win - 1),
                                         skip_group_check=True)
                        lo_c += nl
                        hi_c += nh

                # BN stats all-reduce
                st = per.tile([C, 2], f32, tag=f"stsb{layer}")
                nc.vector.tensor_copy(st[:, 0:1], pstat[:])
                nc.vector.tensor_copy(st[:, 1:2], pstat2[:])
                nc.sync.dma_start(ar_in[layer][:], st[:])
                nc.gpsimd.collective_compute(
                    "AllReduce", OP.add,
                    replica_groups=[list(range(NCORES))],
                    ins=[ar_in[layer].opt()], outs=[ar_out[layer].opt()])
                stg = per.tile([C, 2], f32, tag=f"stg{layer}")
                nc.sync.dma_start(stg[:], ar_out[layer][:])

                mean = per.tile([C, 1], f32, tag=f"mean{layer}")
                nc.vector.tensor_scalar(mean[:], stg[:, 0:1], 1.0 / nfull, None, OP.mult)
                var = per.tile([C, 1], f32, tag=f"var{layer}")
                nc.vector.tensor_scalar(var[:], stg[:, 1:2], 1.0 / nfull, None, OP.mult)
                msq = per.tile([C, 1], f32, tag=f"msq{layer}")
                nc.vector.tensor_tensor(msq[:], mean[:], mean[:], OP.mult)
                nc.vector.tensor_tensor(var[:], var[:], msq[:], OP.subtract)
                nc.vector.tensor_scalar(var[:], var[:], BN_EPS, None, OP.add)
                sd = per.tile([C, 1], f32, tag=f"sd{layer}")
                nc.scalar.activation(sd[:], var[:], AF.Sqrt)
                rstd = per.tile([C, 1], f32, tag=f"rstd{layer}")
                nc.vector.reciprocal(rstd[:], sd[:])
                scl = per.tile([C, 1], f32, tag=f"scl{layer}")
                nc.vector.tensor_tensor(scl[:], gm_sb[layer][:], rstd[:], OP.mult)
                sh = per.tile([C, 1], f32, tag=f"sh{layer}")
                nc.vector.tensor_tensor(sh[:], mean[:], scl[:], OP.mult)
                nc.vector.tensor_tensor(sh[:], bt_sb[layer][:], sh[:], OP.subtract)

                # BN(+ReLU) in transposed domain
                bn_func = AF.Relu if layer == 0 else AF.Identity
                for wi_ in range(win):
                    pT2 = pTp.tile([C, 128], f32, tag="pT")
                    nc.tensor.transpose(pT2[:], h_sb[:, wi_, :], ident[:])
                    nc.scalar.activation(
                        hTn[:, wi_ * 128:(wi_ + 1) * 128], pT2[:],
                        bn_func, bias=sh[:], scale=scl[:])

                # transpose back to node-major
                hn = per.tile([128, win, C], f32, tag="hn", name=f"hn{layer}")
                for wi_ in range(win):
                    pN = pHp.tile([128, C], f32, tag="pH")
                    nc.tensor.matmul(pN[:], hTn[:, wi_ * 128:(wi_ + 1) * 128],
                                     ident[0:C, 0:C], is_transpose=True)
                    nc.vector.tensor_copy(hn[:, wi_, :], pN[:])

                if layer == 0:
                    ag_view = ag_in.opt().rearrange("(w p) c -> p w c", p=128)
                    nc.sync.dma_start(ag_view, hn[:])
                    nc.gpsimd.collective_compute(
                        "AllGather", OP.bypass,
                        replica_groups=[list(range(NCORES))],
                        ins=[ag_in.opt()], outs=[tab1.opt()])
                    hT_prev = hTn
                    if DEBUG_TAPS:
                        nc.sync.dma_start(dbg_h0[:], h_sb[:])
                        nc.sync.dma_start(dbg_hT0[:], hTn[:])
                        nc.sync.dma_start(dbg_tab1[:], tab1.opt())
                        nc.sync.dma_start(dbg_st0[:], stg[:])
                else:
                    out_view = out_h.rearrange("(w p) c -> p w c", p=128)
                    nc.sync.dma_start(out_view, hn[:])

    nc.compile()
    return nc


def make_in_maps(prep, inputs):
    npc, win, node_pad, trows = _derived()
    vals = np.asarray(inputs["vals"], np.float32)
    iota = np.broadcast_to(np.arange(128, dtype=np.float32), (128, 128)).copy()
    ident = np.eye(128, dtype=np.float32)

    tab = np.zeros((trows, C), np.float32)
    for m in range(NCORES):
        tab[m * node_pad:m * node_pad + npc] = vals[m * npc:(m + 1) * npc]

    ncht = prep["ncht"]
    onesv = np.zeros((128, 2), np.float32)
    onesv[:, 0] = 1.0
    tail = npc - (win - 1) * 128
    onesv[:tail, 1] = 1.0

    shared = {"iota": iota, "ident": ident, "onesv": onesv}
    for l in range(2):
        g = np.asarray(inputs[f"g{l}"], np.float32)          # [C, K*C]
        G = np.zeros((K * C, C), np.float32)                 # G[k*C+c, c'] = g[c, k*C+c']
        for k in range(K):
            G[k * C:(k + 1) * C, :] = g[:, k * C:(k + 1) * C]
        shared[f"g{l}c"] = G.reshape(2, 128, C).transpose(1, 0, 2).copy()
        shared[f"rd{l}"] = (np.asarray(inputs[f"root{l}"], np.float32)
                            + np.asarray(inputs[f"dense{l}"], np.float32))
        shared[f"mu{l}r"] = np.broadcast_to(
            np.asarray(inputs[f"mu{l}"], np.float32), (128, K, D)).copy()
        shared[f"sg{l}r"] = np.broadcast_to(
            np.asarray(inputs[f"sigma{l}"], np.float32), (128, K, D)).copy()
        shared[f"gamma{l}"] = np.asarray(inputs[f"gamma{l}"], np.float32).reshape(C, 1)
        shared[f"beta{l}"] = np.asarray(inputs[f"beta{l}"], np.float32).reshape(C, 1)

    in_maps = []
    for m in range(NCORES):
        nslots = ncht * 128
        blk = np.zeros((16, nslots // 16), np.int16)
        s = np.arange(nslots)
        blk[s % 16, s // 16] = prep["idx16"][m]
        idx_w = np.tile(blk, (8, 1))

        ps = np.zeros((128, ncht, D), np.float32)
        ps[s % 128, s // 128] = prep["pseudo_slot"][m]
        dw = np.full((128, ncht), -1.0, np.float32)
        dw[s % 128, s // 128] = prep["dstw"][m]

        xT0 = np.zeros((C, node_pad), np.float32)
        xT0[:, :npc] = vals[m * npc:(m + 1) * npc].T

        in_maps.append(dict(shared, tab=tab, idx=idx_w, pseudo=ps, dstw=dw,
                            xT0=xT0, invdeg=prep["invdeg"][m]))
    return in_maps


_CACHE = {}


def kernel(**inputs):
    global LAST_RESULT
    from concourse.bass_utils import run_bass_kernel_spmd

    npc, win, node_pad, trows = _derived()
    edges = np.asarray(inputs["edges"])
    ekey = hash(edges.tobytes())
    if ekey in _CACHE:
        prep, nc = _CACHE[ekey]
    else:
        prep = host_prep(edges)
        nc = None
    fill_pseudo(prep, inputs["pseudo"])
    if nc is None:
        nc = build_program(prep)
        _CACHE[ekey] = (prep, nc)
    in_maps = make_in_maps(prep, inputs)
    trace = bool(os.environ.get("BASS_KERNEL_TRACE"))
    import time as _time
    _t0 = _time.time()
    res = run_bass_kernel_spmd(nc, in_maps, list(range(NCORES)), trace=trace)
    print(f"[kernel] run_bass_kernel_spmd wall: {_time.time() - _t0:.2f}s", file=sys.stderr)
    LAST_RESULT = res
    out = np.concatenate(
        [res.results[m]["out"][:npc] for m in range(NCORES)], axis=0)
    return np.ascontiguousarray(out, dtype=np.float32)


# revision 18
# speedup vs baseline: 1.8125x; 1.0140x over previous
"""Trainium2 Bass kernel for nn_DenseReluGMMConvNetwork (2-layer GMMConv GNN).

Self-contained: takes FULL inputs, shards nodes across 8 NeuronCores,
runs one SPMD Bass program (gather / GMM weights / scatter-matmul /
BN via AllReduce / inter-layer AllGather), returns FULL [50000, 64] output.
"""

import os
import sys

sys.path.insert(0, "/opt/trn_rl_repo")

import numpy as np

# ---- problem constants (overridable for small-scale sim tests) ----
N = 50000
E = 800000
D = 3
K = 4
C = 64
NCORES = 8
EPS = 1e-15
BN_EPS = 1e-5
SG_WINDOWS = 4          # windows per gather super-group
HALF_SPLIT = 32768      # int16 index range split
DEBUG_TAPS = False

LAST_RESULT = None


def _derived():
    npc = N // NCORES
    win = (npc + 127) // 128          # windows per core
    node_pad = win * 128              # padded rows per core
    trows = NCORES * node_pad         # padded gather-table rows
    return npc, win, node_pad, trows


def host_prep(edges):
    """Route + sort edges, build the uniform chunk grid and slot arrays.

    Returns dict with per-core routed arrays and the static chunk grid.
    """
    npc, win, node_pad, trows = _derived()
    src = np.asarray(edges[0], np.int64)
    dst = np.asarray(edges[1], np.int64)
    e = src.shape[0]

    core = dst // npc
    dl = dst - core * npc
    wi = dl >> 7
    dj = dl & 127
    prow = (src // npc) * node_pad + (src % npc)
    half = (prow >= HALF_SPLIT).astype(np.int64)

    # group key: (core, half, wi); lo region first per core
    gkey = (core * 2 + half) * win + wi
    order = np.argsort(gkey, kind="stable")
    cnt = np.bincount(gkey, minlength=NCORES * 2 * win)
    goff = np.zeros_like(cnt)
    goff[1:] = np.cumsum(cnt)[:-1]
    rank = np.arange(e) - goff[gkey[order]]         # rank within group (sorted order)

    cnt3 = cnt.reshape(NCORES, 2, win)
    ncl = (cnt3[:, 0, :].max(axis=0) + 127) // 128  # lo chunks per window (shared)
    nch = (cnt3[:, 1, :].max(axis=0) + 127) // 128  # hi chunks per window
    ncl = np.maximum(ncl, 1)
    nch = np.maximum(nch, 1)
    nchl_tot = int(ncl.sum())
    nchh_tot = int(nch.sum())
    ncht = nchl_tot + nchh_tot
    lo_off = np.zeros(win, np.int64)
    lo_off[1:] = np.cumsum(ncl)[:-1]
    hi_off = np.zeros(win, np.int64)
    hi_off[1:] = np.cumsum(nch)[:-1]
    hi_off += nchl_tot

    # slot index for each (sorted) edge
    so = order
    chunk_base = np.where(half[so] == 0, lo_off[wi[so]], hi_off[wi[so]])
    slot = chunk_base * 128 + rank

    nslots = ncht * 128
    idx16 = np.zeros((NCORES, nslots), np.int16)
    pseudo_slot = np.zeros((NCORES, nslots, D), np.float32)
    dstw = np.full((NCORES, nslots), -1.0, np.float32)

    cs = core[so]
    iv = prow[so] - half[so] * HALF_SPLIT
    idx16[cs, slot] = iv.astype(np.int16)
    dstw[cs, slot] = dj[so].astype(np.float32)

    deg = np.bincount(dst, minlength=N).astype(np.float32)
    invdeg_flat = 1.0 / np.maximum(deg, 1.0)
    invdeg = np.zeros((NCORES, 128, win), np.float32)
    for m in range(NCORES):
        v = np.zeros(node_pad, np.float32)
        v[:npc] = invdeg_flat[m * npc:(m + 1) * npc]
        invdeg[m] = v.reshape(win, 128).T

    return dict(
        order=order, slot=slot, core_sorted=cs,
        ncl=ncl, nch=nch, ncht=ncht, nchl_tot=nchl_tot,
        lo_off=lo_off, hi_off=hi_off,
        idx16=idx16, pseudo_slot=pseudo_slot, dstw=dstw, invdeg=invdeg,
        npc=npc, win=win, node_pad=node_pad, trows=trows,
    )


def fill_pseudo(prep, pseudo):
    ps = np.asarray(pseudo, np.float32)
    prep["pseudo_slot"][:] = 0.0
    prep["pseudo_slot"][prep["core_sorted"], prep["slot"]] = ps[prep["order"]]


def build_program(prep):
    import concourse.bacc as bacc
    import concourse.mybir as mybir
    import concourse.tile as tile
    from concourse.library_config import mlp

    f32 = mybir.dt.float32
    i16 = mybir.dt.int16
    AF = mybir.ActivationFunctionType
    OP = mybir.AluOpType

    win = prep["win"]
    node_pad = prep["node_pad"]
    trows = prep["trows"]
    ncht = prep["ncht"]
    ncl = prep["ncl"]
    nch = prep["nch"]
    lo_off = prep["lo_off"]
    hi_off = prep["hi_off"]
    nfull = float(N)

    # super-groups of windows
    sgs = [list(range(s, min(s + SG_WINDOWS, win))) for s in range(0, win, SG_WINDOWS)]

    nc = bacc.Bacc("TRN2", target_bir_lowering=False, num_devices=NCORES,
                   num_swdge_queues=4)

    def inp(name, shape, dt=f32):
        return nc.dram_tensor(name, shape, dt, kind="ExternalInput").ap()

    in_tab = inp("tab", [trows, C])
    in_idx = inp("idx", [128, ncht * 8], i16)
    in_ps = inp("pseudo", [128, ncht, D])
    in_dw = inp("dstw", [128, ncht])
    in_xT = inp("xT0", [C, node_pad])
    in_ivd = inp("invdeg", [128, win])
    in_iota = inp("iota", [128, 128])
    in_ident = inp("ident", [128, 128])
    in_ones = inp("onesv", [128, 2])          # col0: ones, col1: valid mask last window
    in_G = [inp(f"g{l}c", [128, 2, C]) for l in range(2)]
    in_RD = [inp(f"rd{l}", [C, C]) for l in range(2)]
    in_mu = [inp(f"mu{l}r", [128, K, D]) for l in range(2)]
    in_sg = [inp(f"sg{l}r", [128, K, D]) for l in range(2)]
    in_gm = [inp(f"gamma{l}", [C, 1]) for l in range(2)]
    in_bt = [inp(f"beta{l}", [C, 1]) for l in range(2)]
    out_h = nc.dram_tensor("out", [node_pad, C], f32, kind="ExternalOutput").ap()
    if DEBUG_TAPS:
        dbg_h0 = nc.dram_tensor("dbg_h0", [128, win, C], f32, kind="ExternalOutput").ap()
        dbg_hT0 = nc.dram_tensor("dbg_hT0", [C, node_pad], f32, kind="ExternalOutput").ap()
        dbg_tab1 = nc.dram_tensor("dbg_tab1", [trows, C], f32, kind="ExternalOutput").ap()
        dbg_st0 = nc.dram_tensor("dbg_st0", [C, 2], f32, kind="ExternalOutput").ap()

    with tile.TileContext(nc) as tc:
        nc.gpsimd.load_library(mlp)
        with tc.tile_pool(name="const", bufs=1) as cpool, \
             tc.tile_pool(name="sg", bufs=2) as sgp, \
             tc.tile_pool(name="wn", bufs=2) as wnp, \
             tc.tile_pool(name="per", bufs=1) as per, \
             tc.tile_pool(name="pB", bufs=2, space="PSUM") as pBp, \
             tc.tile_pool(name="pT", bufs=2, space="PSUM") as pTp, \
             tc.tile_pool(name="pH", bufs=2, space="PSUM") as pHp, \
             tc.tile_pool(name="pS", bufs=1, space="PSUM") as pSp, \
             tc.tile_pool(name="dram", bufs=1, space="DRAM") as dram:

            iota = cpool.tile([128, 128], f32)
            nc.sync.dma_start(iota[:], in_iota[:])
            ident = cpool.tile([128, 128], f32)
            nc.sync.dma_start(ident[:], in_ident[:])
            ones = cpool.tile([128, 2], f32)
            nc.sync.dma_start(ones[:], in_ones[:])
            ivd = cpool.tile([128, win], f32)
            nc.sync.dma_start(ivd[:], in_ivd[:])

            G_sb, RD_sb, gm_sb, bt_sb, quad = [], [], [], [], []
            for l in range(2):
                g_t = cpool.tile([128, 2, C], f32, tag=f"G{l}")
                nc.sync.dma_start(g_t[:], in_G[l][:])
                G_sb.append(g_t)
                rd_t = cpool.tile([C, C], f32, tag=f"RD{l}")
                nc.sync.dma_start(rd_t[:], in_RD[l][:])
                RD_sb.append(rd_t)
                gm_t = cpool.tile([C, 1], f32, tag=f"gm{l}")
                nc.sync.dma_start(gm_t[:], in_gm[l][:])
                gm_sb.append(gm_t)
                bt_t = cpool.tile([C, 1], f32, tag=f"bt{l}")
                nc.sync.dma_start(bt_t[:], in_bt[l][:])
                bt_sb.append(bt_t)
                mu_t = cpool.tile([128, K, D], f32, tag=f"mu{l}")
                nc.sync.dma_start(mu_t[:], in_mu[l][:])
                sg_t = cpool.tile([128, K, D], f32, tag=f"sg{l}")
                nc.sync.dma_start(sg_t[:], in_sg[l][:])
                # inv2s = 0.5 / (EPS + sigma^2), computed once on device
                s2 = cpool.tile([128, K, D], f32, tag=f"s2{l}")
                nc.vector.tensor_tensor(s2[:], sg_t[:], sg_t[:], OP.mult)
                nc.vector.tensor_scalar(s2[:], s2[:], EPS, None, OP.add)
                nc.vector.reciprocal(s2[:], s2[:])
                nc.vector.tensor_scalar(s2[:], s2[:], 0.5, None, OP.mult)
                quad.append((mu_t, s2))

            tab1 = dram.tile([trows, C], f32)
            ag_in = dram.tile([node_pad, C], f32)
            ar_in = [dram.tile([C, 2], f32, tag=f"ari{l}", name=f"ari{l}") for l in range(2)]
            ar_out = [dram.tile([C, 2], f32, tag=f"aro{l}", name=f"aro{l}") for l in range(2)]

            hT_prev = None
            for layer in range(2):
                tab_ap = in_tab if layer == 0 else tab1.opt()
                mu_t, inv_t = quad[layer]
                h_sb = per.tile([128, win, C], f32, tag="h", name=f"h{layer}")
                hTn = per.tile([C, node_pad], f32, tag=f"hT{layer}")
                pstat = pSp.tile([C, 1], f32, tag="st")
                pstat2 = pSp.tile([C, 1], f32, tag="st2")

                gq = [0]
                for sg_wins in sgs:
                    w0, wE = sg_wins[0], sg_wins[-1]
                    nlo = int(ncl[w0:wE + 1].sum())
                    nhi = int(nch[w0:wE + 1].sum())
                    nsg = nlo + nhi
                    clo0 = int(lo_off[w0])
                    chi0 = int(hi_off[w0])

                    xg = sgp.tile([128, nsg, C], f32, tag="xg")
                    idxs = sgp.tile([128, nsg * 8], i16, tag="idxs", bufs=3)
                    ps_t = sgp.tile([128, nsg, D], f32, tag="ps", bufs=3)
                    dw_t = sgp.tile([128, nsg], f32, tag="dw", bufs=3)

                    nc.sync.dma_start(idxs[:, :nlo * 8],
                                      in_idx[:, clo0 * 8:(clo0 + nlo) * 8])
                    nc.sync.dma_start(idxs[:, nlo * 8:],
                                      in_idx[:, chi0 * 8:(chi0 + nhi) * 8])
                    nc.sync.dma_start(ps_t[:, :nlo, :], in_ps[:, clo0:clo0 + nlo, :])
                    nc.sync.dma_start(ps_t[:, nlo:, :], in_ps[:, chi0:chi0 + nhi, :])
                    nc.sync.dma_start(dw_t[:, :nlo], in_dw[:, clo0:clo0 + nlo])
                    nc.sync.dma_start(dw_t[:, nlo:], in_dw[:, chi0:chi0 + nhi])

                    # split gathers at GMAX idxs: single_packet coalesces each
                    # engine's descs into ONE packet (<=64 descs/engine spec cap
                    # => <=1024 idxs per instruction; larger wedges the SDMA)
                    GMAX = 8  # chunks per gather instruction (8*128 = 1024 idx)
                    for (base, n, tview) in (
                            (0, nlo, tab_ap[0:HALF_SPLIT, :]),
                            (nlo, nhi, tab_ap[HALF_SPLIT:trows, :])):
                        for g0 in range(0, n, GMAX):
                            gn = min(GMAX, n - g0)
                            c0 = base + g0
                            nc.gpsimd.dma_gather(
                                xg[:, c0:c0 + gn, :], tview,
                                idxs[:, c0 * 8:(c0 + gn) * 8],
                                gn * 128, gn * 128, C,
                                queue_num=gq[0] % 4)
                            gq[0] += 1

                    # GMM weights: w[p, j, k] = exp(-sum_d inv2s*(ps - mu)^2)
                    dif = sgp.tile([128, nsg, K, D], f32, tag="dif")
                    nc.vector.tensor_tensor(
                        dif[:],
                        ps_t[:].unsqueeze(2).broadcast_to((128, nsg, K, D)),
                        mu_t[:].unsqueeze(1).broadcast_to((128, nsg, K, D)),
                        OP.subtract)
                    nc.vector.tensor_tensor(dif[:], dif[:], dif[:], OP.mult)
                    nc.vector.tensor_tensor(
                        dif[:], dif[:],
                        inv_t[:].unsqueeze(1).broadcast_to((128, nsg, K, D)),
                        OP.mult)
                    zt = sgp.tile([128, nsg, K], f32, tag="zt", bufs=3)
                    nc.vector.tensor_reduce(zt[:], dif[:], mybir.AxisListType.X, OP.add)
                    w_t = sgp.tile([128, nsg, K], f32, tag="wt", bufs=3)
                    nc.scalar.activation(w_t[:], zt[:], AF.Exp, scale=-1.0)

                    lo_c = 0
                    hi_c = nlo
                    for wi_ in sg_wins:
                        nl = int(ncl[wi_])
                        nh = int(nch[wi_])
                        ncw = nl + nh
                        ranges = [(lo_c, nl, 0), (hi_c, nh, nl)]

                        S = wnp.tile([128, ncw, 128], f32, tag="S")
                        xw = wnp.tile([128, ncw, K, C], f32, tag="xw")
                        for (c0, n, o) in ranges:
                            nc.vector.tensor_tensor(
                                S[:, o:o + n, :],
                                dw_t[:, c0:c0 + n].unsqueeze(2).broadcast_to((128, n, 128)),
                                iota[:].unsqueeze(1).broadcast_to((128, n, 128)),
                                OP.is_equal)
                            # xw split: k=0..2 on VectorE, k=3 on ScalarE
                            # (per-partition scale) to unblock the DVE-bound
                            # per-window pipeline
                            nc.vector.tensor_tensor(
                                xw[:, o:o + n, 0:3, :],
                                xg[:, c0:c0 + n, :].unsqueeze(2).broadcast_to((128, n, 3, C)),
                                w_t[:, c0:c0 + n, 0:3].unsqueeze(3).broadcast_to((128, n, 3, C)),
                                OP.mult)
                            for j in range(n):
                                nc.scalar.activation(
                                    xw[:, o + j, 3, :], xg[:, c0 + j, :],
                                    AF.Copy, scale=w_t[:, c0 + j, 3:4])

                        pB = pBp.tile([128, K * C], f32, tag="pB")
                        for cj in range(ncw):
                            nc.tensor.matmul(
                                pB[:], S[:, cj, :], xw[:, cj, :, :].opt(),
                                start=(cj == 0), stop=(cj == ncw - 1))

                        bsb = wnp.tile([128, K * C], f32, tag="bsb")
                        nc.vector.tensor_scalar(
                            bsb[:], pB[:], ivd[:, wi_:wi_ + 1], None, OP.mult)

                        pT = pTp.tile([128, K * C], f32, tag="pT")
                        nc.tensor.transpose(pT[:, 0:128], bsb[:, 0:128], ident[:])
                        nc.tensor.transpose(pT[:, 128:256], bsb[:, 128:256], ident[:])
                        bT = wnp.tile([128, K * C], f32, tag="bT")
                        nc.vector.tensor_copy(bT[:], pT[:])

                        if layer == 0:
                            xTw = wnp.tile([C, 128], f32, tag="xTw")
                            nc.sync.dma_start(
                                xTw[:], in_xT[:, wi_ * 128:(wi_ + 1) * 128])
                            xT_ap = xTw[:]
                        else:
                            xT_ap = hT_prev[:, wi_ * 128:(wi_ + 1) * 128]

                        pH = pHp.tile([128, C], f32, tag="pH")
                        nc.tensor.matmul(pH[:], bT[:, 0:128], G_sb[layer][:, 0, :],
                                         start=True, stop=False)
                        nc.tensor.matmul(pH[:], bT[:, 128:256], G_sb[layer][:, 1, :],
                                         start=False, stop=False)
                        nc.tensor.matmul(pH[:], xT_ap, RD_sb[layer][:],
                                         start=False, stop=True)

                        nc.vector.tensor_copy(h_sb[:, wi_, :], pH[:])
                        hsq = wnp.tile([128, C], f32, tag="hsq")
                        nc.scalar.activation(hsq[:], h_sb[:, wi_, :], AF.Square)
                        mcol = 1 if wi_ == win - 1 else 0
                        nc.tensor.matmul(pstat[:], h_sb[:, wi_, :],
                                         ones[:, mcol:mcol + 1],
                                         start=(wi_ == 0), stop=(wi_ == win - 1),
                                         skip_group_check=True)
                        nc.tensor.matmul(pstat2[:], hsq[:],
                                         ones[:, mcol:mcol + 1],
                                         start=(wi_ == 0), stop=(wi_ == win - 1),
                                         skip_group_check=True)
                        lo_c += nl
                        hi_c += nh

                # BN stats all-reduce
                st = per.tile([C, 2], f32, tag=f"stsb{layer}")
                nc.vector.tensor_copy(st[:, 0:1], pstat[:])
                nc.vector.tensor_copy(st[:, 1:2], pstat2[:])
                nc.sync.dma_start(ar_in[layer][:], st[:])
                nc.gpsimd.collective_compute(
                    "AllReduce", OP.add,
                    replica_groups=[list(range(NCORES))],
                    ins=[ar_in[layer].opt()], outs=[ar_out[layer].opt()])
                stg = per.tile([C, 2], f32, tag=f"stg{layer}")
                nc.sync.dma_start(stg[:], ar_out[layer][:])

                mean = per.tile([C, 1], f32, tag=f"mean{layer}")
                nc.vector.tensor_scalar(mean[:], stg[:, 0:1], 1.0 / nfull, None, OP.mult)
                var = per.tile([C, 1], f32, tag=f"var{layer}")
                nc.vector.tensor_scalar(var[:], stg[:, 1:2], 1.0 / nfull, None, OP.mult)
                msq = per.tile([C, 1], f32, tag=f"msq{layer}")
                nc.vector.tensor_tensor(msq[:], mean[:], mean[:], OP.mult)
                nc.vector.tensor_tensor(var[:], var[:], msq[:], OP.subtract)
                nc.vector.tensor_scalar(var[:], var[:], BN_EPS, None, OP.add)
                sd = per.tile([C, 1], f32, tag=f"sd{layer}")
                nc.scalar.activation(sd[:], var[:], AF.Sqrt)
                rstd = per.tile([C, 1], f32, tag=f"rstd{layer}")
                nc.vector.reciprocal(rstd[:], sd[:])
                scl = per.tile([C, 1], f32, tag=f"scl{layer}")
                nc.vector.tensor_tensor(scl[:], gm_sb[layer][:], rstd[:], OP.mult)
                sh = per.tile([C, 1], f32, tag=f"sh{layer}")
                nc.vector.tensor_tensor(sh[:], mean[:], scl[:], OP.mult)
                nc.vector.tensor_tensor(sh[:], bt_sb[layer][:], sh[:], OP.subtract)

                # BN(+ReLU) in transposed domain
                bn_func = AF.Relu if layer == 0 else AF.Identity
                for wi_ in range(win):
                    pT2 = pTp.tile([C, 128], f32, tag="pT")
                    nc.tensor.transpose(pT2[:], h_sb[:, wi_, :], ident[:])
                    nc.scalar.activation(
                        hTn[:, wi_ * 128:(wi_ + 1) * 128], pT2[:],
                        bn_func, bias=sh[:], scale=scl[:])

                # transpose back to node-major
                hn = per.tile([128, win, C], f32, tag="hn", name=f"hn{layer}")
                for wi_ in range(win):
                    pN = pHp.tile([128, C], f32, tag="pH")
                    nc.tensor.matmul(pN[:], hTn[:, wi_ * 128:(wi_ + 1) * 128],
                                     ident[0:C, 0:C], is_transpose=True)
                    nc.vector.tensor_copy(hn[:, wi_, :], pN[:])

                if layer == 0:
                    ag_view = ag_in.opt().rearrange("(w p) c -> p w c", p=128)
                    nc.sync.dma_start(ag_view, hn[:])
                    nc.gpsimd.collective_compute(
                        "AllGather", OP.bypass,
                        replica_groups=[list(range(NCORES))],
                        ins=[ag_in.opt()], outs=[tab1.opt()])
                    hT_prev = hTn
                    if DEBUG_TAPS:
                        nc.sync.dma_start(dbg_h0[:], h_sb[:])
                        nc.sync.dma_start(dbg_hT0[:], hTn[:])
                        nc.sync.dma_start(dbg_tab1[:], tab1.opt())
                        nc.sync.dma_start(dbg_st0[:], stg[:])
                else:
                    out_view = out_h.rearrange("(w p) c -> p w c", p=128)
                    nc.sync.dma_start(out_view, hn[:])

    nc.compile()
    return nc


def make_in_maps(prep, inputs):
    npc, win, node_pad, trows = _derived()
    vals = np.asarray(inputs["vals"], np.float32)
    iota = np.broadcast_to(np.arange(128, dtype=np.float32), (128, 128)).copy()
    ident = np.eye(128, dtype=np.float32)

    tab = np.zeros((trows, C), np.float32)
    for m in range(NCORES):
        tab[m * node_pad:m * node_pad + npc] = vals[m * npc:(m + 1) * npc]

    ncht = prep["ncht"]
    onesv = np.zeros((128, 2), np.float32)
    onesv[:, 0] = 1.0
    tail = npc - (win - 1) * 128
    onesv[:tail, 1] = 1.0

    shared = {"iota": iota, "ident": ident, "onesv": onesv}
    for l in range(2):
        g = np.asarray(inputs[f"g{l}"], np.float32)          # [C, K*C]
        G = np.zeros((K * C, C), np.float32)                 # G[k*C+c, c'] = g[c, k*C+c']
        for k in range(K):
            G[k * C:(k + 1) * C, :] = g[:, k * C:(k + 1) * C]
        shared[f"g{l}c"] = G.reshape(2, 128, C).transpose(1, 0, 2).copy()
        shared[f"rd{l}"] = (np.asarray(inputs[f"root{l}"], np.float32)
                            + np.asarray(inputs[f"dense{l}"], np.float32))
        shared[f"mu{l}r"] = np.broadcast_to(
            np.asarray(inputs[f"mu{l}"], np.float32), (128, K, D)).copy()
        shared[f"sg{l}r"] = np.broadcast_to(
            np.asarray(inputs[f"sigma{l}"], np.float32), (128, K, D)).copy()
        shared[f"gamma{l}"] = np.asarray(inputs[f"gamma{l}"], np.float32).reshape(C, 1)
        shared[f"beta{l}"] = np.asarray(inputs[f"beta{l}"], np.float32).reshape(C, 1)

    in_maps = []
    for m in range(NCORES):
        nslots = ncht * 128
        blk = np.zeros((16, nslots // 16), np.int16)
        s = np.arange(nslots)
        blk[s % 16, s // 16] = prep["idx16"][m]
        idx_w = np.tile(blk, (8, 1))

        ps = np.zeros((128, ncht, D), np.float32)
        ps[s % 128, s // 128] = prep["pseudo_slot"][m]
        dw = np.full((128, ncht), -1.0, np.float32)
        dw[s % 128, s // 128] = prep["dstw"][m]

        xT0 = np.zeros((C, node_pad), np.float32)
        xT0[:, :npc] = vals[m * npc:(m + 1) * npc].T

        in_maps.append(dict(shared, tab=tab, idx=idx_w, pseudo=ps, dstw=dw,
                            xT0=xT0, invdeg=prep["invdeg"][m]))
    return in_maps


_CACHE = {}


def kernel(**inputs):
    global LAST_RESULT
    from concourse.bass_utils import run_bass_kernel_spmd

    npc, win, node_pad, trows = _derived()
    edges = np.asarray(inputs["edges"])
    ekey = hash(edges.tobytes())
    if ekey in _CACHE:
        prep, nc = _CACHE[ekey]
    else:
        prep = host_prep(edges)
        nc = None
    fill_pseudo(prep, inputs["pseudo"])
    if nc is None:
        nc = build_program(prep)
        _CACHE[ekey] = (prep, nc)
    in_maps = make_in_maps(prep, inputs)
    trace = bool(os.environ.get("BASS_KERNEL_TRACE"))
    import time as _time
    _t0 = _time.time()
    res = run_bass_kernel_spmd(nc, in_maps, list(range(NCORES)), trace=trace)
    print(f"[kernel] run_bass_kernel_spmd wall: {_time.time() - _t0:.2f}s", file=sys.stderr)
    LAST_RESULT = res
    out = np.concatenate(
        [res.results[m]["out"][:npc] for m in range(NCORES)], axis=0)
    return np.ascontiguousarray(out, dtype=np.float32)
